# revision 37
# baseline (speedup 1.0000x reference)
"""Trainium2 Bass kernel for nn_Attention_16011638079620 (gnn_message_passing).

Computes, for feats [8192, 256] f32 and kn=10:
    sim   = cosine-similarity(feats)            [N, N]
    B     = rowwise top-kn one-hot mask of softmax(sim) (rank-preserving)
    G     = (1/kn) * invdv_i * invdv_j * (B^T B)_ij,  dv = colsums of B

Strategy (8 cores):
  - sim via 3-pass bf16 hi/lo split matmuls (exact top-k, 4x faster than f32)
  - B columns packed 3-per-fp16 value (base 24; counts <= 23 so the packed
    matmul B^T @ packedB is integer-exact in fp32 PSUM) -> 1.5x fp8-DR rate
  - G is symmetric: core c computes blocks (c, (c+d)%8) for d=0..4; host
    mirrors the rest. lhsT is always the core's own column slice (AllToAll).
  - dv via fp8-DoubleRow ones-matmuls, interleaved with the sim phase.
  - software pipeline: mask/pack of block m-1 runs behind sim of block m so
    PSUM banks release early; collectives merged in block pairs.
"""

import sys

sys.path.insert(0, "/opt/trn_rl_repo")

from contextlib import ExitStack

import numpy as np

import concourse.bass as bass
import concourse.tile as tile
from concourse import bacc, mybir
from concourse.bass import _add_dep_helper
from concourse.bass_utils import run_bass_kernel_spmd

f32 = mybir.dt.float32
bf16 = mybir.dt.bfloat16
fp16 = mybir.dt.float16
fp8 = mybir.dt.float8e4
i32 = mybir.dt.int32
Alu = mybir.AluOpType
Act = mybir.ActivationFunctionType
NEG = -1e30
BASE = 24.0
B2 = BASE * BASE  # 576
MAGIC = 12582912.0  # 1.5 * 2**23: (z + MAGIC) - MAGIC == round-to-nearest(z)
CB2 = 0.5 / B2 - 0.5  # bias so round(P/576 + CB2) == floor(P/576) exactly
CB1 = 0.5 / BASE - 0.5


def build_nc(N, D, KN, NCORES):
    RP = N // NCORES           # 1024 rows/G-rows per core
    MB = RP // 128             # 8 row blocks per core
    NCH = N // 512             # topk chunks
    DT = D // 128              # 2 feature chunks
    PC = 342                   # packed cols per 1024-col slice (342*3 = 1026)
    PCW = NCORES * PC          # 2736 packed cols total per row
    PAD = 3 * PC               # 1026 padded cols per slice
    BW = NCORES * PAD          # 8208 padded mask width
    KK = N // 128              # 64 contraction chunks for phase D
    NP = MB // 2               # 4 block pairs for collectives
    ND = 5                     # symmetric blocks per core
    assert 8 < KN <= 16

    inv_de = float(np.float32(1.0) / np.float32(KN))

    nc = bacc.Bacc(
        "TRN2",
        target_bir_lowering=False,
        debug=False,
        enable_asserts=False,
        num_devices=NCORES,
    )
    feats_all = nc.dram_tensor("feats_all", [N, D], f32, kind="ExternalInput").ap()
    feats_my = nc.dram_tensor("feats_my", [RP, D], f32, kind="ExternalInput").ap()
    ident_in = nc.dram_tensor("ident_in", [128, 128], f32, kind="ExternalInput").ap()
    jsel_in = nc.dram_tensor("jsel_in", [1, 8], i32, kind="ExternalInput").ap()
    g_out = nc.dram_tensor("g_out", [RP, ND * RP], f32, kind="ExternalOutput").ap()

    rg = [list(range(NCORES))]

    with tile.TileContext(nc) as tc, ExitStack() as ctx:
        dram = ctx.enter_context(tc.tile_pool(name="dram", bufs=1, space="DRAM"))
        b_grp2 = [
            dram.tile([NCORES, 2, 128, RP], fp8, name=f"b_grp2_{p}") for p in range(NP)
        ]
        lts_d2 = [
            dram.tile([NCORES, 2, 128, RP], fp8, name=f"lts_d2_{p}") for p in range(NP)
        ]
        # per-dest packed payload: 5 consecutive column slices (dest needs js c..c+4)
        # pk_ext echoes slices 0..3 after slice 7 so every dest window is
        # one contiguous run; payload is then built DRAM->DRAM so the sim-side
        # SBUF ring releases immediately
        pk_ext = [
            dram.tile([2, 128, PCW + (ND - 1) * PC], fp16, name=f"pk_ext_{p}")
            for p in range(NP)
        ]
        pk5_in2 = [
            dram.tile([NCORES, 2, 128, ND * PC], fp16, name=f"pk5_in2_{p}")
            for p in range(NP)
        ]
        pk5_ag2 = [
            dram.tile([NCORES, 2, 128, ND * PC], fp16, name=f"pk5_ag2_{p}")
            for p in range(NP)
        ]
        dv_my_d = dram.tile([RP], f32, name="dv_my_d")
        dv_full = dram.tile([N], f32, addr_space="Shared", name="dv_full")
        cs_dram = dram.tile([N], f32, name="cs_dram")

        pers = ctx.enter_context(tc.tile_pool(name="pers", bufs=1))
        dv_ps = None  # allocated from g_ps during the G d0 window

        ident = pers.tile([128, 128], f32, name="ident")
        nc.sync.dma_start(ident[:], ident_in)
        idb = pers.tile([128, 128], bf16, name="idb")
        nc.vector.tensor_copy(idb[:], ident[:])
        jsel_sb = pers.tile([1, 8], i32, name="jsel_sb")
        nc.sync.dma_start(jsel_sb[:], jsel_in)
        ones1 = pers.tile([1, 128], f32, name="ones1")
        nc.vector.memset(ones1[:], 1.0)
        zlh = pers.tile([1, 128], fp16, name="zlh")
        nc.vector.memset(zlh[:], 0.0)
        zrh = pers.tile([1, 512], fp16, name="zrh")
        nc.vector.memset(zrh[:], 0.0)
        ones_dr = pers.tile([128, 2, 16], fp8, name="ones_dr")
        nc.vector.memset(ones_dr[:], 1.0)

        # lhsT for phase D: [128, slot = m'*NCORES+o, my 1024 cols] fp8 (8MB)
        lt_all = pers.tile([128, KK, RP], fp8, name="lt_all")

        # ---------------- phase 1: normalize + hi/lo split + transpose ------
        with ExitStack() as p12:
            fsb = p12.enter_context(tc.tile_pool(name="fsb", bufs=1))
            fnt_hi = [fsb.tile([128, N], bf16, name=f"fh{h}") for h in range(DT)]
            fnt_lo = [fsb.tile([128, N], bf16, name=f"fl{h}") for h in range(DT)]
            fnt_myh = [fsb.tile([128, RP], bf16, name=f"fmh{h}") for h in range(DT)]
            fnt_myl = [fsb.tile([128, RP], bf16, name=f"fml{h}") for h in range(DT)]

            with ExitStack() as p1:
                wrk = p1.enter_context(tc.tile_pool(name="wrk", bufs=3))
                sml = p1.enter_context(tc.tile_pool(name="sml", bufs=6))
                tp_ps = p1.enter_context(
                    tc.tile_pool(name="tp_ps", bufs=2, space="PSUM")
                )

                def norm_group(src4, dh, dl, col0, nb):
                    # nb row-blocks batched: one op set for the whole group
                    ft4 = wrk.tile([128, nb, D], f32, name="ft4")
                    nc.sync.dma_start(ft4[:], src4)
                    tps = {}
                    for x in range(2):
                        tps[x] = tp_ps.tile(
                            [128, DT, nb * 128], bf16, name=f"tp{x}", tag=f"tp{x}"
                        )
                    sq4 = wrk.tile([128, nb, D], f32, name="sq4")
                    nc.scalar.square(
                        sq4.rearrange("p b d -> p (b d)"),
                        ft4.rearrange("p b d -> p (b d)"),
                    )
                    n24 = sml.tile([128, nb, 1], f32, name="n24")
                    nc.vector.reduce_sum(n24[:], sq4[:], axis=mybir.AxisListType.X)
                    nrm4 = sml.tile([128, nb, 1], f32, name="nrm4")
                    nc.scalar.sqrt(
                        nrm4.rearrange("p b o -> p (b o)"),
                        n24.rearrange("p b o -> p (b o)"),
                    )
                    inv4 = sml.tile([128, nb, 1], f32, name="inv4")
                    nc.vector.reciprocal(
                        inv4.rearrange("p b o -> p (b o)"),
                        nrm4.rearrange("p b o -> p (b o)"),
                    )
                    fn4 = wrk.tile([128, nb, D], f32, name="fn4")
                    nc.vector.tensor_tensor(
                        fn4[:], ft4[:], inv4[:].broadcast_to([128, nb, D]),
                        op=Alu.mult,
                    )
                    fh4 = wrk.tile([128, nb, D], bf16, name="fh4")
                    nc.scalar.copy(
                        fh4.rearrange("p b d -> p (b d)"),
                        fn4.rearrange("p b d -> p (b d)"),
                    )
                    fl4 = wrk.tile([128, nb, D], bf16, name="fl4")
                    nc.vector.tensor_tensor(
                        fl4[:], fn4[:], fh4[:], op=Alu.subtract
                    )
                    for i in range(nb):
                        for h in range(DT):
                            for x, s4 in ((0, fh4), (1, fl4)):
                                nc.tensor.transpose(
                                    tps[x][:, h, i * 128 : (i + 1) * 128],
                                    s4[:, i, h * 128 : (h + 1) * 128],
                                    idb[:],
                                )
                    for h in range(DT):
                        for x, dst in ((0, dh), (1, dl)):
                            nc.scalar.copy(
                                dst[h][:, col0 : col0 + nb * 128], tps[x][:, h, :]
                            )

                fm4 = feats_my.rearrange("(g i p) d -> g p i d", p=128, i=4)
                for g in range(MB // 4):
                    norm_group(fm4[g], fnt_myh, fnt_myl, g * 512, 4)
                fa4 = feats_all.rearrange("(g i p) d -> g p i d", p=128, i=4)
                for g in range(N // 512):
                    norm_group(fa4[g], fnt_hi, fnt_lo, g * 512, 4)

            # ---------------- phase 2: sim, topk, mask, pack, CC -----------
            with ExitStack() as p2:
                simp = p2.enter_context(tc.tile_pool(name="simp", bufs=3))
                smal = p2.enter_context(tc.tile_pool(name="smal", bufs=2))
                bmpp = p2.enter_context(tc.tile_pool(name="bmpp", bufs=1))
                pkp = p2.enter_context(tc.tile_pool(name="pkp", bufs=1))
                t0p = p2.enter_context(tc.tile_pool(name="t0p", bufs=2))
                sim_ps = p2.enter_context(
                    tc.tile_pool(name="sim_ps", bufs=1, space="PSUM")
                )
                combos = []
                for h in range(DT):
                    combos.append((fnt_myh[h], fnt_hi[h]))
                    combos.append((fnt_myh[h], fnt_lo[h]))
                    combos.append((fnt_myl[h], fnt_hi[h]))

                tkns = {}
                halves = {}

                def sim_block(m):
                    sh0 = simp.tile([128, N // 2], f32, name="sh0", tag="sh")
                    sh1 = simp.tile([128, N // 2], f32, name="sh1", tag="sh")
                    halves[m] = (sh0, sh1)
                    cand = smal.tile([128, 8 * NCH], f32, name="cand", tag="cand")
                    for qr in range(4):
                        pss = [
                            sim_ps.tile([128, 512], f32, name=f"sq{t}", tag=f"sq{t}")
                            for t in range(4)
                        ]
                        for ci, (la, ra) in enumerate(combos):
                            lt = la[:, m * 128 : (m + 1) * 128]
                            for t in range(4):
                                ntc = qr * 4 + t
                                nc.tensor.matmul(
                                    pss[t][:],
                                    lt,
                                    ra[:, ntc * 512 : (ntc + 1) * 512],
                                    start=(ci == 0),
                                    stop=(ci == 5),
                                )
                        sh = (sh0, sh1)[qr // 2]
                        for t in range(4):
                            ntc = qr * 4 + t
                            nc.vector.max(
                                cand[:, ntc * 8 : (ntc + 1) * 8], pss[t][:]
                            )
                            nc.scalar.copy(
                                sh[:, (ntc % 8) * 512 : (ntc % 8 + 1) * 512],
                                pss[t][:],
                            )
                    c8 = smal.tile([128, 8], f32, name="c8", tag="c8")
                    nc.vector.max(c8[:], cand[:])
                    cand2 = smal.tile([128, 8 * NCH], f32, name="cand2", tag="cand2")
                    nc.vector.match_replace(cand2[:], c8[:], cand[:], NEG)
                    c8b = smal.tile([128, 8], f32, name="c8b", tag="c8b")
                    nc.vector.max(c8b[:], cand2[:])
                    tkns[m] = c8b

                def mask_pack(m):
                    tkn = tkns[m][:, KN - 9 : KN - 8]
                    sh0, sh1 = halves[m]
                    bmp = bmpp.tile([128, BW], fp8, name="bmp")
                    for j in range(NCORES):
                        sh = (sh0, sh1)[j // 4]
                        nc.vector.tensor_scalar(
                            bmp[:, j * PAD : j * PAD + RP],
                            sh[:, (j % 4) * RP : (j % 4 + 1) * RP],
                            tkn,
                            None,
                            op0=Alu.is_ge,
                        )
                        nc.vector.memset(bmp[:, j * PAD + RP : (j + 1) * PAD], 0.0)
                    pk = pkp.tile([128, PCW], fp16, name="pk")
                    bm3 = bmp.rearrange("p (j t u) -> p j t u", j=NCORES, u=3)
                    for j in range(NCORES):
                        t0 = t0p.tile([128, PC], f32, name="t0")
                        nc.vector.scalar_tensor_tensor(
                            t0[:], in0=bm3[:, j, :, 1], scalar=BASE,
                            in1=bm3[:, j, :, 0], op0=Alu.mult, op1=Alu.add,
                        )
                        nc.vector.scalar_tensor_tensor(
                            pk[:, j * PC : (j + 1) * PC],
                            in0=bm3[:, j, :, 2], scalar=B2, in1=t0[:],
                            op0=Alu.mult, op1=Alu.add,
                        )
                    pp, i = m // 2, m % 2
                    nc.sync.dma_start(pk_ext[pp][i, :, 0:PCW], pk[:])
                    nc.sync.dma_start(
                        pk_ext[pp][i, :, PCW : PCW + (ND - 1) * PC],
                        pk[:, 0 : (ND - 1) * PC],
                    )
                    for c in range(NCORES):
                        nc.sync.dma_start(
                            pk5_in2[pp][c, i],
                            pk_ext[pp][i, :, c * PC : (c + ND) * PC],
                        )
                    nc.sync.dma_start(
                        b_grp2[pp][:, i].rearrange("j p q -> p j q"),
                        bmp.rearrange("p (j q) -> p j q", j=NCORES)[:, :, 0:RP],
                    )

                lt_dma = {}

                def cc_pk5(pp):
                    nc.gpsimd.collective_compute(
                        "AllToAll", Alu.bypass, replica_groups=rg,
                        ins=[pk5_in2[pp].opt()], outs=[pk5_ag2[pp].opt()],
                    )

                def cc_pair(pp, defer_pk=False):
                    # small lt A2A first so dv/G lhs unblocks earliest.
                    # lt_dma is NOT issued here: a descriptor waiting on the
                    # A2A semaphore head-of-line-blocks its DMA ring and
                    # stalls the sim-phase payload writes queued behind it.
                    nc.gpsimd.collective_compute(
                        "AllToAll", Alu.bypass, replica_groups=rg,
                        ins=[b_grp2[pp].opt()], outs=[lts_d2[pp].opt()],
                    )
                    if not defer_pk:
                        cc_pk5(pp)

                ltp = lt_all.rearrange("p (t i o) q -> p t i o q", i=2, o=NCORES)

                def dv_pair(pp):
                    first = True
                    for o in range(NCORES):
                        for m in range(MB):
                            mm = nc.tensor.matmul(
                                dv_ps[:, m : m + 1],
                                ltp[:, pp, :, o, m * 128 : (m + 1) * 128],
                                ones_dr[:, :, 0:1],
                                perf_mode=mybir.MatmulPerfMode.DoubleRow,
                                start=(pp == 0 and o == 0 and m == 0),
                                stop=(pp == NP - 1 and o == NCORES - 1 and m == MB - 1),
                                skip_group_check=True,
                            )
                            if first:
                                for mp in (2 * pp, 2 * pp + 1):
                                    _add_dep_helper(
                                        mm.ins, lt_dma[mp].ins, sync=True,
                                        reason="dv after lt load",
                                    )
                                first = False

                for m in range(MB):
                    sim_block(m)
                    if m >= 1:
                        mask_pack(m - 1)
                        if (m - 1) % 2 == 1:
                            cc_pair((m - 1) // 2)
                mask_pack(MB - 1)
                # defer pair-3 pk5 so the dv AllGather can jump the CC queue
                cc_pair(NP - 1, defer_pk=True)
                lt_dmas = lt_dma
                dv_pair_fn = dv_pair
                cc_pk5_fn = cc_pk5

        # ---------------- phase 3+4: pair-major G, SBUF accumulation --------
        # G units (pair, d, m) run as each pair's collectives land, so PE
        # work rides inside the CC windows; per-(d,m) psum partials are
        # added into SBUF accumulators and only pair-3's sweep + the output
        # stage remain after the last arrival.
        G0 = [(m * 128) // 3 for m in range(MB)]
        with ExitStack() as p4:
            gw = p4.enter_context(tc.tile_pool(name="gw", bufs=4))
            csp = p4.enter_context(tc.tile_pool(name="csp", bufs=2))
            stg = p4.enter_context(tc.tile_pool(name="stg", bufs=2))
            esp = p4.enter_context(tc.tile_pool(name="esp", bufs=1))
            accp = p4.enter_context(tc.tile_pool(name="accp", bufs=1))
            rhp = p4.enter_context(tc.tile_pool(name="rhp", bufs=2))
            g_ps = p4.enter_context(tc.tile_pool(name="g_ps", bufs=1, space="PSUM"))

            acc = {
                d: accp.tile([128, MB, PC], f32, name=f"acc{d}")
                for d in range(ND)
            }

            def los_of(d):
                return G0 if d in (0, ND - 1) else [0] * MB

            # lt loads issued post-sim so their collective waits don't block
            # sim-era DMA rings
            for mp in range(MB):
                lt_dmas[mp] = nc.sync.dma_start(
                    lt_all[:, mp * NCORES : (mp + 1) * NCORES, :],
                    lts_d2[mp // 2][:, mp % 2].rearrange("o p q -> p o q"),
                )

            def load_rh5(pp, d):
                rh = rhp.tile([128, 2, NCORES, PC], fp16, name="rh5", tag="rh5")
                for i in range(2):
                    nc.sync.dma_start(
                        rh[:, i],
                        pk5_ag2[pp][:, i, :, d * PC : (d + 1) * PC]
                        .rearrange("o p c -> p o c"),
                    )
                return rh

            dv_ps = g_ps.tile([128, 512], f32, name="dv_ps", tag="gp7")
            uc = 0
            for pp in range(NP):
                # dv first: pair-3's dv only needs lt3, so the dv AllGather
                # can run ahead of the pair-3 pk5 A2A in the CC stream
                dv_pair_fn(pp)
                if pp == NP - 1:
                    dv_sb = gw.tile([128, MB], f32, name="dv_sb")
                    nc.vector.tensor_copy(dv_sb[:], dv_ps[:, 0:MB])
                    nc.scalar.dma_start(
                        dv_my_d.rearrange("(m p) -> p m", p=128), dv_sb[:]
                    )
                    nc.gpsimd.collective_compute(
                        "AllGather", Alu.bypass, replica_groups=rg,
                        ins=[dv_my_d.opt()], outs=[dv_full.opt()],
                    )
                    cc_pk5_fn(NP - 1)
                rhs = {}
                rhs[0] = load_rh5(pp, 0)
                for d in range(ND):
                    if d + 1 < ND:
                        rhs[d + 1] = load_rh5(pp, d + 1)
                    los = los_of(d)
                    rh = rhs.pop(d)
                    for m in range(MB):
                        lo = los[m]
                        ps = g_ps.tile(
                            [128, 512], f32, name="gps", tag=f"gp{uc % 7}"
                        )
                        uc += 1
                        k = 0
                        for i in range(2):
                            for o in range(NCORES):
                                s = (2 * pp + i) * NCORES + o
                                mm = nc.tensor.matmul(
                                    ps[:, 0 : PC - lo],
                                    lt_all[:, s, m * 128 : (m + 1) * 128],
                                    rh[:, i, o, lo:PC],
                                    start=(k == 0),
                                    stop=(k == 15),
                                    skip_group_check=True,
                                )
                                if k == 0 and m == 0 and d == 0:
                                    for ii in range(2):
                                        _add_dep_helper(
                                            mm.ins,
                                            lt_dmas[2 * pp + ii].ins,
                                            sync=True, reason="G after lt",
                                        )
                                k += 1
                        asl = acc[d][:, m, lo:PC]
                        if pp == 0:
                            nc.scalar.copy(asl, ps[:, 0 : PC - lo])
                        else:
                            nc.vector.tensor_tensor(
                                asl, asl, ps[:, 0 : PC - lo], op=Alu.add
                            )

            # ---- scales (dv is complete) --------------------------------
            d1 = gw.tile([128, MB], f32, name="d1")
            nc.vector.tensor_scalar_max(d1[:], dv_sb[:], 1.0)
            sq = gw.tile([128, MB], f32, name="sqv")
            nc.scalar.sqrt(sq[:], d1[:])
            rc = gw.tile([128, MB], f32, name="rc")
            nc.vector.reciprocal(rc[:], sq[:])
            mk = gw.tile([128, MB], f32, name="mk")
            nc.vector.tensor_scalar(mk[:], dv_sb[:], 0.0, None, op0=Alu.is_gt)
            iv = gw.tile([128, MB], f32, name="iv")
            nc.vector.tensor_tensor(iv[:], rc[:], mk[:], op=Alu.mult)
            rs0 = gw.tile([128, MB], f32, name="rsu0")
            nc.vector.tensor_scalar_mul(rs0[:], iv[:], inv_de)

            q = N // 128
            dvw = gw.tile([128, q], f32, name="dvw")
            nc.scalar.dma_start(dvw[:], dv_full.rearrange("(cm p) -> p cm", p=128))
            d1w = gw.tile([128, q], f32, name="d1w")
            nc.vector.tensor_scalar_max(d1w[:], dvw[:], 1.0)
            sqw = gw.tile([128, q], f32, name="sqw")
            nc.scalar.sqrt(sqw[:], d1w[:])
            rcw = gw.tile([128, q], f32, name="rcw")
            nc.vector.reciprocal(rcw[:], sqw[:])
            mkw = gw.tile([128, q], f32, name="mkw")
            nc.vector.tensor_scalar(mkw[:], dvw[:], 0.0, None, op0=Alu.is_gt)
            ivw = gw.tile([128, q], f32, name="ivw")
            nc.vector.tensor_tensor(ivw[:], rcw[:], mkw[:], op=Alu.mult)
            nc.scalar.dma_start(cs_dram.rearrange("(cm p) -> p cm", p=128), ivw[:])
            cs2d = cs_dram.rearrange("(a n) -> a n", a=1)
            csts = {}
            for dd in range(ND):
                jra = nc.scalar.alloc_register(f"jra{dd}")
                nc.scalar.reg_load(jra, jsel_sb[0:1, dd : dd + 1])
                jda = nc.scalar.snap(
                    jra, donate=True, min_val=0, max_val=NCORES - 1
                )
                cs_t = gw.tile([1, RP], f32, name="cs_t", tag=f"cs_t{dd % 2}")
                nc.scalar.dma_start(cs_t[:], cs2d[:, bass.ds(jda * RP, RP)])
                csts[dd] = cs_t

            # ---- output stage: unpack digits, scale, store --------------
            for d in range(ND):
                los = los_of(d)
                csj = csp.tile([128, PAD], f32, name="csj", tag="csj")
                for hh in range(2):
                    cps = g_ps.tile(
                        [128, 512], f32, name="cps", tag=f"gp{uc % 7}"
                    )
                    uc += 1
                    nc.tensor.matmul(
                        cps[:], ones1[:],
                        csts[d][:, hh * 512 : (hh + 1) * 512],
                        start=True, stop=True,
                    )
                    nc.scalar.copy(csj[:, hh * 512 : (hh + 1) * 512], cps[:])
                nc.vector.memset(csj[:, RP:PAD], 0.0)
                csj3 = csj.rearrange("p (t u) -> p t u", u=3)
                for m in range(MB):
                    lo = los[m]
                    es = acc[d][:, m, :]
                    z2 = esp.tile([128, PC], f32, name="z2", tag="z2")
                    nc.vector.tensor_scalar(
                        z2[:, lo:], es[:, lo:], 1.0 / B2, CB2,
                        op0=Alu.mult, op1=Alu.add,
                    )
                    m2 = esp.tile([128, PC], f32, name="m2", tag="m2")
                    nc.vector.tensor_scalar(
                        m2[:, lo:], z2[:, lo:], MAGIC, MAGIC,
                        op0=Alu.add, op1=Alu.subtract,
                    )
                    r2 = esp.tile([128, PC], f32, name="r2", tag="r2")
                    nc.vector.scalar_tensor_tensor(
                        r2[:, lo:], in0=m2[:, lo:], scalar=-B2, in1=es[:, lo:],
                        op0=Alu.mult, op1=Alu.add,
                    )
                    z1 = esp.tile([128, PC], f32, name="z1", tag="z1")
                    nc.vector.tensor_scalar(
                        z1[:, lo:], r2[:, lo:], 1.0 / BASE, CB1,
                        op0=Alu.mult, op1=Alu.add,
                    )
                    m1 = esp.tile([128, PC], f32, name="m1", tag="m1")
                    nc.vector.tensor_scalar(
                        m1[:, lo:], z1[:, lo:], MAGIC, MAGIC,
                        op0=Alu.add, op1=Alu.subtract,
                    )
                    m0 = esp.tile([128, PC], f32, name="m0", tag="m0")
                    nc.vector.scalar_tensor_tensor(
                        m0[:, lo:], in0=m1[:, lo:], scalar=-BASE, in1=r2[:, lo:],
                        op0=Alu.mult, op1=Alu.add,
                    )
                    gs = stg.tile([128, PAD], f32, name="gs", tag="gs")
                    gs3 = gs.rearrange("p (t u) -> p t u", u=3)
                    for u, mu in ((2, m2), (1, m1), (0, m0)):
                        au = esp.tile([128, PC], f32, name=f"a{u}", tag="au")
                        nc.scalar.activation(
                            au[:, lo:], mu[:, lo:], Act.Copy,
                            scale=rs0[:, m : m + 1],
                        )
                        nc.vector.tensor_tensor(
                            gs3[:, lo:, u], au[:, lo:], csj3[:, lo:, u],
                            op=Alu.mult,
                        )
                    nc.sync.dma_start(
                        g_out[
                            m * 128 : (m + 1) * 128,
                            d * RP + 3 * lo : (d + 1) * RP,
                        ],
                        gs[:, 3 * lo : RP],
                    )

    nc.compile()
    return nc


_CACHE = {}


def get_nc(N, D, KN, NCORES):
    key = (N, D, KN, NCORES)
    if key not in _CACHE:
        _CACHE[key] = build_nc(N, D, KN, NCORES)
    return _CACHE[key]


def kernel(feats, kn, _trace=False):
    feats = np.asarray(feats, dtype=np.float32)
    kn = int(kn)
    N, D = feats.shape
    NCORES = 8
    ND = 5
    RP = N // NCORES
    nc = get_nc(N, D, kn, NCORES)
    ident = np.eye(128, dtype=np.float32)
    in_maps = []
    for c in range(NCORES):
        jsel = np.zeros((1, 8), np.int32)
        for d in range(8):
            jsel[0, d] = (c + d) % NCORES
        in_maps.append({
            "feats_all": feats,
            "feats_my": feats[c * RP : (c + 1) * RP],
            "ident_in": ident,
            "jsel_in": jsel,
        })
    res = run_bass_kernel_spmd(
        nc, in_maps, core_ids=list(range(NCORES)), trace=_trace
    )
    out = np.empty((N, N), dtype=np.float32)
    res_g = [res.results[c]["g_out"] for c in range(NCORES)]  # [RP, ND*RP]
    for c in range(NCORES):
        g = res_g[c]
        for d in range(ND):
            j = (c + d) % NCORES
            blk = g[:, d * RP : (d + 1) * RP]
            if d == 0:
                # kernel computed upper triangle only; mirror below diag
                full = np.triu(blk) + np.triu(blk, 1).T
                out[c * RP : (c + 1) * RP, c * RP : (c + 1) * RP] = full
            elif d == ND - 1:
                if c >= NCORES // 2:
                    continue
                # core c has triu of (c,j); core j has triu of (j,c) = tril^T
                blk2 = res_g[j][:, (ND - 1) * RP : ND * RP]
                full = np.triu(blk) + np.tril(blk2.T, -1)
                out[c * RP : (c + 1) * RP, j * RP : (j + 1) * RP] = full
                out[j * RP : (j + 1) * RP, c * RP : (c + 1) * RP] = full.T
            else:
                out[c * RP : (c + 1) * RP, j * RP : (j + 1) * RP] = blk
                out[j * RP : (j + 1) * RP, c * RP : (c + 1) * RP] = blk.T
    if _trace:
        return out, res
    return out


if __name__ == "__main__":
    inputs = {
        "feats": np.load("/tmp/feats.npy"),
        "kn": 10,
    }
    out = kernel(**inputs)
    print("out", out.shape, out.dtype, float(np.abs(out).max()))



# revision 43
# speedup vs baseline: 1.3490x; 1.3490x over previous
"""Trainium2 Bass kernel for nn_Attention_16011638079620 (gnn_message_passing).

Computes, for feats [8192, 256] f32 and kn=10:
    sim   = cosine-similarity(feats)            [N, N]
    B     = rowwise top-kn one-hot mask of softmax(sim) (rank-preserving)
    G     = (1/kn) * invdv_i * invdv_j * (B^T B)_ij,  dv = colsums of B

Strategy (8 cores):
  - sim via 3-pass bf16 hi/lo split matmuls (exact top-k, 4x faster than f32)
  - B columns packed 3-per-fp16 value (base 24; counts <= 23 so the packed
    matmul B^T @ packedB is integer-exact in fp32 PSUM) -> 1.5x fp8-DR rate
  - G is symmetric: core c computes blocks (c, (c+d)%8) for d=0..4; host
    mirrors the rest. lhsT is always the core's own column slice (AllToAll).
  - dv via fp8-DoubleRow ones-matmuls, interleaved with the sim phase.
  - software pipeline: mask/pack of block m-1 runs behind sim of block m so
    PSUM banks release early; collectives merged in block pairs.
"""

import sys

sys.path.insert(0, "/opt/trn_rl_repo")

from contextlib import ExitStack

import numpy as np

import concourse.bass as bass
import concourse.tile as tile
from concourse import bacc, mybir
from concourse.bass import _add_dep_helper
from concourse.bass_utils import run_bass_kernel_spmd

f32 = mybir.dt.float32
bf16 = mybir.dt.bfloat16
fp16 = mybir.dt.float16
fp8 = mybir.dt.float8e4
i32 = mybir.dt.int32
Alu = mybir.AluOpType
Act = mybir.ActivationFunctionType
NEG = -1e30
BASE = 24.0
B2 = BASE * BASE  # 576
MAGIC = 12582912.0  # 1.5 * 2**23: (z + MAGIC) - MAGIC == round-to-nearest(z)
CB2 = 0.5 / B2 - 0.5  # bias so round(P/576 + CB2) == floor(P/576) exactly
CB1 = 0.5 / BASE - 0.5


def build_nc(N, D, KN, NCORES):
    RP = N // NCORES           # 1024 rows/G-rows per core
    MB = RP // 128             # 8 row blocks per core
    NCH = N // 512             # topk chunks
    DT = D // 128              # 2 feature chunks
    PC = 342                   # packed cols per 1024-col slice (342*3 = 1026)
    PCW = NCORES * PC          # 2736 packed cols total per row
    PAD = 3 * PC               # 1026 padded cols per slice
    BW = NCORES * PAD          # 8208 padded mask width
    KK = N // 128              # 64 contraction chunks for phase D
    NP = MB // 2               # 4 block pairs for collectives
    ND = 5                     # symmetric blocks per core
    assert 8 < KN <= 16

    inv_de = float(np.float32(1.0) / np.float32(KN))

    nc = bacc.Bacc(
        "TRN2",
        target_bir_lowering=False,
        debug=False,
        enable_asserts=False,
        num_devices=NCORES,
    )
    feats_all = nc.dram_tensor("feats_all", [N, D], f32, kind="ExternalInput").ap()
    feats_my = nc.dram_tensor("feats_my", [RP, D], f32, kind="ExternalInput").ap()
    ident_in = nc.dram_tensor("ident_in", [128, 128], f32, kind="ExternalInput").ap()
    jsel_in = nc.dram_tensor("jsel_in", [1, 8], i32, kind="ExternalInput").ap()
    g_out = nc.dram_tensor("g_out", [RP, ND * RP], f32, kind="ExternalOutput").ap()

    rg = [list(range(NCORES))]

    with tile.TileContext(nc) as tc, ExitStack() as ctx:
        dram = ctx.enter_context(tc.tile_pool(name="dram", bufs=1, space="DRAM"))
        b_grp2 = [
            dram.tile([NCORES, 2, 128, RP], fp8, name=f"b_grp2_{p}") for p in range(NP)
        ]
        lts_d2 = [
            dram.tile([NCORES, 2, 128, RP], fp8, name=f"lts_d2_{p}") for p in range(NP)
        ]
        # pk exchange is an AllGather: one cheap contiguous write per block
        # (A2A variants need 5x duplicated payload staging, which drowns the
        # sync engine in DIRECT2D descriptor work and stalls the sim phase)
        pk_in2 = [dram.tile([2, 128, PCW], fp16, name=f"pk_in2_{p}") for p in range(NP)]
        pk_ag2 = [
            dram.tile(
                [NCORES, 2, 128, PCW], fp16, addr_space="Shared", name=f"pk_ag2_{p}"
            )
            for p in range(NP)
        ]
        dv_my_d = dram.tile([RP], f32, name="dv_my_d")
        dv_full = dram.tile([N], f32, addr_space="Shared", name="dv_full")
        cs_dram = dram.tile([N], f32, name="cs_dram")

        pers = ctx.enter_context(tc.tile_pool(name="pers", bufs=1))
        dv_ps = None  # allocated from g_ps during the G d0 window

        ident = pers.tile([128, 128], f32, name="ident")
        nc.sync.dma_start(ident[:], ident_in)
        idb = pers.tile([128, 128], bf16, name="idb")
        nc.vector.tensor_copy(idb[:], ident[:])
        jsel_sb = pers.tile([1, 8], i32, name="jsel_sb")
        nc.sync.dma_start(jsel_sb[:], jsel_in)
        ones1 = pers.tile([1, 128], f32, name="ones1")
        nc.vector.memset(ones1[:], 1.0)
        zlh = pers.tile([1, 128], fp16, name="zlh")
        nc.vector.memset(zlh[:], 0.0)
        zrh = pers.tile([1, 512], fp16, name="zrh")
        nc.vector.memset(zrh[:], 0.0)
        ones_dr = pers.tile([128, 2, 16], fp8, name="ones_dr")
        nc.vector.memset(ones_dr[:], 1.0)

        # lhsT for phase D: [128, slot = m'*NCORES+o, my 1024 cols] fp8 (8MB)
        lt_all = pers.tile([128, KK, RP], fp8, name="lt_all")

        # ---------------- phase 1: normalize + hi/lo split + transpose ------
        with ExitStack() as p12:
            fsb = p12.enter_context(tc.tile_pool(name="fsb", bufs=1))
            fnt_hi = [fsb.tile([128, N], bf16, name=f"fh{h}") for h in range(DT)]
            fnt_lo = [fsb.tile([128, N], bf16, name=f"fl{h}") for h in range(DT)]
            fnt_myh = [fsb.tile([128, RP], bf16, name=f"fmh{h}") for h in range(DT)]
            fnt_myl = [fsb.tile([128, RP], bf16, name=f"fml{h}") for h in range(DT)]

            with ExitStack() as p1:
                wrk = p1.enter_context(tc.tile_pool(name="wrk", bufs=3))
                sml = p1.enter_context(tc.tile_pool(name="sml", bufs=6))
                tp_ps = p1.enter_context(
                    tc.tile_pool(name="tp_ps", bufs=2, space="PSUM")
                )

                def norm_group(src4, dh, dl, col0, nb):
                    # nb row-blocks batched: one op set for the whole group
                    ft4 = wrk.tile([128, nb, D], f32, name="ft4")
                    nc.sync.dma_start(ft4[:], src4)
                    tps = {}
                    for x in range(2):
                        tps[x] = tp_ps.tile(
                            [128, DT, nb * 128], bf16, name=f"tp{x}", tag=f"tp{x}"
                        )
                    sq4 = wrk.tile([128, nb, D], f32, name="sq4")
                    nc.scalar.square(
                        sq4.rearrange("p b d -> p (b d)"),
                        ft4.rearrange("p b d -> p (b d)"),
                    )
                    n24 = sml.tile([128, nb, 1], f32, name="n24")
                    nc.vector.reduce_sum(n24[:], sq4[:], axis=mybir.AxisListType.X)
                    nrm4 = sml.tile([128, nb, 1], f32, name="nrm4")
                    nc.scalar.sqrt(
                        nrm4.rearrange("p b o -> p (b o)"),
                        n24.rearrange("p b o -> p (b o)"),
                    )
                    inv4 = sml.tile([128, nb, 1], f32, name="inv4")
                    nc.vector.reciprocal(
                        inv4.rearrange("p b o -> p (b o)"),
                        nrm4.rearrange("p b o -> p (b o)"),
                    )
                    fn4 = wrk.tile([128, nb, D], f32, name="fn4")
                    nc.vector.tensor_tensor(
                        fn4[:], ft4[:], inv4[:].broadcast_to([128, nb, D]),
                        op=Alu.mult,
                    )
                    fh4 = wrk.tile([128, nb, D], bf16, name="fh4")
                    nc.scalar.copy(
                        fh4.rearrange("p b d -> p (b d)"),
                        fn4.rearrange("p b d -> p (b d)"),
                    )
                    fl4 = wrk.tile([128, nb, D], bf16, name="fl4")
                    nc.vector.tensor_tensor(
                        fl4[:], fn4[:], fh4[:], op=Alu.subtract
                    )
                    for i in range(nb):
                        for h in range(DT):
                            for x, s4 in ((0, fh4), (1, fl4)):
                                nc.tensor.transpose(
                                    tps[x][:, h, i * 128 : (i + 1) * 128],
                                    s4[:, i, h * 128 : (h + 1) * 128],
                                    idb[:],
                                )
                    for h in range(DT):
                        for x, dst in ((0, dh), (1, dl)):
                            nc.scalar.copy(
                                dst[h][:, col0 : col0 + nb * 128], tps[x][:, h, :]
                            )

                fm4 = feats_my.rearrange("(g i p) d -> g p i d", p=128, i=4)
                for g in range(MB // 4):
                    norm_group(fm4[g], fnt_myh, fnt_myl, g * 512, 4)
                fa4 = feats_all.rearrange("(g i p) d -> g p i d", p=128, i=4)
                for g in range(N // 512):
                    norm_group(fa4[g], fnt_hi, fnt_lo, g * 512, 4)

            # ---------------- phase 2: sim, topk, mask, pack, CC -----------
            with ExitStack() as p2:
                simp = p2.enter_context(tc.tile_pool(name="simp", bufs=3))
                smal = p2.enter_context(tc.tile_pool(name="smal", bufs=2))
                bmpp = p2.enter_context(tc.tile_pool(name="bmpp", bufs=1))
                pkp = p2.enter_context(tc.tile_pool(name="pkp", bufs=1))
                t0p = p2.enter_context(tc.tile_pool(name="t0p", bufs=2))
                sim_ps = p2.enter_context(
                    tc.tile_pool(name="sim_ps", bufs=1, space="PSUM")
                )
                combos = []
                for h in range(DT):
                    combos.append((fnt_myh[h], fnt_hi[h]))
                    combos.append((fnt_myh[h], fnt_lo[h]))
                    combos.append((fnt_myl[h], fnt_hi[h]))

                tkns = {}
                halves = {}

                def sim_block(m):
                    sh0 = simp.tile([128, N // 2], f32, name="sh0", tag="sh")
                    sh1 = simp.tile([128, N // 2], f32, name="sh1", tag="sh")
                    halves[m] = (sh0, sh1)
                    cand = smal.tile([128, 8 * NCH], f32, name="cand", tag="cand")
                    for qr in range(4):
                        pss = [
                            sim_ps.tile([128, 512], f32, name=f"sq{t}", tag=f"sq{t}")
                            for t in range(4)
                        ]
                        for ci, (la, ra) in enumerate(combos):
                            lt = la[:, m * 128 : (m + 1) * 128]
                            for t in range(4):
                                ntc = qr * 4 + t
                                nc.tensor.matmul(
                                    pss[t][:],
                                    lt,
                                    ra[:, ntc * 512 : (ntc + 1) * 512],
                                    start=(ci == 0),
                                    stop=(ci == 5),
                                )
                        sh = (sh0, sh1)[qr // 2]
                        for t in range(4):
                            ntc = qr * 4 + t
                            nc.vector.max(
                                cand[:, ntc * 8 : (ntc + 1) * 8], pss[t][:]
                            )
                            nc.scalar.copy(
                                sh[:, (ntc % 8) * 512 : (ntc % 8 + 1) * 512],
                                pss[t][:],
                            )
                    c8 = smal.tile([128, 8], f32, name="c8", tag="c8")
                    nc.vector.max(c8[:], cand[:])
                    cand2 = smal.tile([128, 8 * NCH], f32, name="cand2", tag="cand2")
                    nc.vector.match_replace(cand2[:], c8[:], cand[:], NEG)
                    c8b = smal.tile([128, 8], f32, name="c8b", tag="c8b")
                    nc.vector.max(c8b[:], cand2[:])
                    tkns[m] = c8b

                def mask_pack(m):
                    tkn = tkns[m][:, KN - 9 : KN - 8]
                    sh0, sh1 = halves[m]
                    bmp = bmpp.tile([128, BW], fp8, name="bmp")
                    for j in range(NCORES):
                        sh = (sh0, sh1)[j // 4]
                        nc.vector.tensor_scalar(
                            bmp[:, j * PAD : j * PAD + RP],
                            sh[:, (j % 4) * RP : (j % 4 + 1) * RP],
                            tkn,
                            None,
                            op0=Alu.is_ge,
                        )
                        nc.vector.memset(bmp[:, j * PAD + RP : (j + 1) * PAD], 0.0)
                    pk = pkp.tile([128, PCW], fp16, name="pk")
                    bm3 = bmp.rearrange("p (j t u) -> p j t u", j=NCORES, u=3)
                    for j in range(NCORES):
                        t0 = t0p.tile([128, PC], f32, name="t0")
                        nc.vector.scalar_tensor_tensor(
                            t0[:], in0=bm3[:, j, :, 1], scalar=BASE,
                            in1=bm3[:, j, :, 0], op0=Alu.mult, op1=Alu.add,
                        )
                        nc.vector.scalar_tensor_tensor(
                            pk[:, j * PC : (j + 1) * PC],
                            in0=bm3[:, j, :, 2], scalar=B2, in1=t0[:],
                            op0=Alu.mult, op1=Alu.add,
                        )
                    pp, i = m // 2, m % 2
                    nc.sync.dma_start(pk_in2[pp][i], pk[:])
                    nc.sync.dma_start(
                        b_grp2[pp][:, i].rearrange("j p q -> p j q"),
                        bmp.rearrange("p (j q) -> p j q", j=NCORES)[:, :, 0:RP],
                    )

                lt_dma = {}

                def cc_pk5(pp):
                    nc.gpsimd.collective_compute(
                        "AllGather", Alu.bypass, replica_groups=rg,
                        ins=[pk_in2[pp].opt()], outs=[pk_ag2[pp].opt()],
                    )

                def cc_pair(pp, defer_pk=False):
                    # small lt A2A first so dv/G lhs unblocks earliest.
                    # lt_dma is NOT issued here: a descriptor waiting on the
                    # A2A semaphore head-of-line-blocks its DMA ring and
                    # stalls the sim-phase payload writes queued behind it.
                    nc.gpsimd.collective_compute(
                        "AllToAll", Alu.bypass, replica_groups=rg,
                        ins=[b_grp2[pp].opt()], outs=[lts_d2[pp].opt()],
                    )
                    if not defer_pk:
                        cc_pk5(pp)

                ltp = lt_all.rearrange("p (t i o) q -> p t i o q", i=2, o=NCORES)

                def dv_pair(pp):
                    first = True
                    for o in range(NCORES):
                        for m in range(MB):
                            mm = nc.tensor.matmul(
                                dv_ps[:, m : m + 1],
                                ltp[:, pp, :, o, m * 128 : (m + 1) * 128],
                                ones_dr[:, :, 0:1],
                                perf_mode=mybir.MatmulPerfMode.DoubleRow,
                                start=(pp == 0 and o == 0 and m == 0),
                                stop=(pp == NP - 1 and o == NCORES - 1 and m == MB - 1),
                                skip_group_check=True,
                            )
                            if first:
                                for mp in (2 * pp, 2 * pp + 1):
                                    _add_dep_helper(
                                        mm.ins, lt_dma[mp].ins, sync=True,
                                        reason="dv after lt load",
                                    )
                                first = False

                for m in range(MB):
                    sim_block(m)
                    if m >= 1:
                        mask_pack(m - 1)
                        if (m - 1) % 2 == 1:
                            cc_pair((m - 1) // 2)
                mask_pack(MB - 1)
                # defer pair-3 pk5 so the dv AllGather can jump the CC queue
                cc_pair(NP - 1, defer_pk=True)
                lt_dmas = lt_dma
                dv_pair_fn = dv_pair
                cc_pk5_fn = cc_pk5

        # ---------------- phase 3+4: pair-major G, SBUF accumulation --------
        # G units (pair, d, m) run as each pair's collectives land, so PE
        # work rides inside the CC windows; per-(d,m) psum partials are
        # added into SBUF accumulators and only pair-3's sweep + the output
        # stage remain after the last arrival.
        G0 = [(m * 128) // 3 for m in range(MB)]
        with ExitStack() as p4:
            gw = p4.enter_context(tc.tile_pool(name="gw", bufs=4))
            csp = p4.enter_context(tc.tile_pool(name="csp", bufs=2))
            stg = p4.enter_context(tc.tile_pool(name="stg", bufs=2))
            esp = p4.enter_context(tc.tile_pool(name="esp", bufs=1))
            accp = p4.enter_context(tc.tile_pool(name="accp", bufs=1))
            rhp = p4.enter_context(tc.tile_pool(name="rhp", bufs=2))
            g_ps = p4.enter_context(tc.tile_pool(name="g_ps", bufs=1, space="PSUM"))

            acc = {
                d: accp.tile([128, MB, PC], f32, name=f"acc{d}")
                for d in range(ND)
            }

            def los_of(d):
                return G0 if d in (0, ND - 1) else [0] * MB

            # lt loads are issued per pair inside the pair loop: the sync
            # queue is in-order, so a descriptor waiting on a late collective
            # must not sit ahead of earlier pairs' rh loads

            jds = {}
            for dd in range(ND):
                jr = nc.sync.alloc_register(f"jr{dd}")
                nc.sync.reg_load(jr, jsel_sb[0:1, dd : dd + 1])
                jds[dd] = nc.sync.snap(
                    jr, donate=True, min_val=0, max_val=NCORES - 1
                )

            def load_rh5(pp, d):
                rh = rhp.tile([128, 2, NCORES, PC], fp16, name="rh5", tag="rh5")
                for i in range(2):
                    nc.sync.dma_start(
                        rh[:, i],
                        pk_ag2[pp][:, i, :, :].rearrange("o p c -> p o c")[
                            :, :, bass.ds(jds[d] * PC, PC)
                        ],
                    )
                return rh

            dv_ps = g_ps.tile([128, 512], f32, name="dv_ps", tag="gp7")
            uc = 0
            for pp in range(NP):
                for ii in range(2):
                    mp = 2 * pp + ii
                    lt_dmas[mp] = nc.sync.dma_start(
                        lt_all[:, mp * NCORES : (mp + 1) * NCORES, :],
                        lts_d2[pp][:, ii].rearrange("o p q -> p o q"),
                    )
                # dv first: pair-3's dv only needs lt3, so the dv AllGather
                # can run ahead of the pair-3 pk AllGather in the CC stream
                dv_pair_fn(pp)
                if pp == NP - 1:
                    dv_sb = gw.tile([128, MB], f32, name="dv_sb")
                    nc.vector.tensor_copy(dv_sb[:], dv_ps[:, 0:MB])
                    nc.scalar.dma_start(
                        dv_my_d.rearrange("(m p) -> p m", p=128), dv_sb[:]
                    )
                    nc.gpsimd.collective_compute(
                        "AllGather", Alu.bypass, replica_groups=rg,
                        ins=[dv_my_d.opt()], outs=[dv_full.opt()],
                    )
                    cc_pk5_fn(NP - 1)
                rhs = {}
                rhs[0] = load_rh5(pp, 0)
                for d in range(ND):
                    if d + 1 < ND:
                        rhs[d + 1] = load_rh5(pp, d + 1)
                    los = los_of(d)
                    rh = rhs.pop(d)
                    for m in range(MB):
                        lo = los[m]
                        ps = g_ps.tile(
                            [128, 512], f32, name="gps", tag=f"gp{uc % 7}"
                        )
                        uc += 1
                        k = 0
                        for i in range(2):
                            for o in range(NCORES):
                                s = (2 * pp + i) * NCORES + o
                                mm = nc.tensor.matmul(
                                    ps[:, 0 : PC - lo],
                                    lt_all[:, s, m * 128 : (m + 1) * 128],
                                    rh[:, i, o, lo:PC],
                                    start=(k == 0),
                                    stop=(k == 15),
                                    skip_group_check=True,
                                )
                                if k == 0 and m == 0 and d == 0:
                                    for ii in range(2):
                                        _add_dep_helper(
                                            mm.ins,
                                            lt_dmas[2 * pp + ii].ins,
                                            sync=True, reason="G after lt",
                                        )
                                k += 1
                        asl = acc[d][:, m, lo:PC]
                        if pp == 0:
                            nc.scalar.copy(asl, ps[:, 0 : PC - lo])
                        else:
                            nc.vector.tensor_tensor(
                                asl, asl, ps[:, 0 : PC - lo], op=Alu.add
                            )

            # ---- scales (dv is complete) --------------------------------
            d1 = gw.tile([128, MB], f32, name="d1")
            nc.vector.tensor_scalar_max(d1[:], dv_sb[:], 1.0)
            sq = gw.tile([128, MB], f32, name="sqv")
            nc.scalar.sqrt(sq[:], d1[:])
            rc = gw.tile([128, MB], f32, name="rc")
            nc.vector.reciprocal(rc[:], sq[:])
            mk = gw.tile([128, MB], f32, name="mk")
            nc.vector.tensor_scalar(mk[:], dv_sb[:], 0.0, None, op0=Alu.is_gt)
            iv = gw.tile([128, MB], f32, name="iv")
            nc.vector.tensor_tensor(iv[:], rc[:], mk[:], op=Alu.mult)
            rs0 = gw.tile([128, MB], f32, name="rsu0")
            nc.vector.tensor_scalar_mul(rs0[:], iv[:], inv_de)

            q = N // 128
            dvw = gw.tile([128, q], f32, name="dvw")
            nc.scalar.dma_start(dvw[:], dv_full.rearrange("(cm p) -> p cm", p=128))
            d1w = gw.tile([128, q], f32, name="d1w")
            nc.vector.tensor_scalar_max(d1w[:], dvw[:], 1.0)
            sqw = gw.tile([128, q], f32, name="sqw")
            nc.scalar.sqrt(sqw[:], d1w[:])
            rcw = gw.tile([128, q], f32, name="rcw")
            nc.vector.reciprocal(rcw[:], sqw[:])
            mkw = gw.tile([128, q], f32, name="mkw")
            nc.vector.tensor_scalar(mkw[:], dvw[:], 0.0, None, op0=Alu.is_gt)
            ivw = gw.tile([128, q], f32, name="ivw")
            nc.vector.tensor_tensor(ivw[:], rcw[:], mkw[:], op=Alu.mult)
            nc.scalar.dma_start(cs_dram.rearrange("(cm p) -> p cm", p=128), ivw[:])
            cs2d = cs_dram.rearrange("(a n) -> a n", a=1)
            csts = {}
            for dd in range(ND):
                jra = nc.scalar.alloc_register(f"jra{dd}")
                nc.scalar.reg_load(jra, jsel_sb[0:1, dd : dd + 1])
                jda = nc.scalar.snap(
                    jra, donate=True, min_val=0, max_val=NCORES - 1
                )
                cs_t = gw.tile([1, RP], f32, name="cs_t", tag=f"cs_t{dd % 2}")
                nc.scalar.dma_start(cs_t[:], cs2d[:, bass.ds(jda * RP, RP)])
                csts[dd] = cs_t

            # ---- output stage: unpack digits, scale, store --------------
            for d in range(ND):
                los = los_of(d)
                csj = csp.tile([128, PAD], f32, name="csj", tag="csj")
                for hh in range(2):
                    cps = g_ps.tile(
                        [128, 512], f32, name="cps", tag=f"gp{uc % 7}"
                    )
                    uc += 1
                    nc.tensor.matmul(
                        cps[:], ones1[:],
                        csts[d][:, hh * 512 : (hh + 1) * 512],
                        start=True, stop=True,
                    )
                    nc.scalar.copy(csj[:, hh * 512 : (hh + 1) * 512], cps[:])
                nc.vector.memset(csj[:, RP:PAD], 0.0)
                csj3 = csj.rearrange("p (t u) -> p t u", u=3)
                for m in range(MB):
                    lo = los[m]
                    es = acc[d][:, m, :]
                    z2 = esp.tile([128, PC], f32, name="z2", tag="z2")
                    nc.vector.tensor_scalar(
                        z2[:, lo:], es[:, lo:], 1.0 / B2, CB2,
                        op0=Alu.mult, op1=Alu.add,
                    )
                    m2 = esp.tile([128, PC], f32, name="m2", tag="m2")
                    nc.vector.tensor_scalar(
                        m2[:, lo:], z2[:, lo:], MAGIC, MAGIC,
                        op0=Alu.add, op1=Alu.subtract,
                    )
                    r2 = esp.tile([128, PC], f32, name="r2", tag="r2")
                    nc.vector.scalar_tensor_tensor(
                        r2[:, lo:], in0=m2[:, lo:], scalar=-B2, in1=es[:, lo:],
                        op0=Alu.mult, op1=Alu.add,
                    )
                    z1 = esp.tile([128, PC], f32, name="z1", tag="z1")
                    nc.vector.tensor_scalar(
                        z1[:, lo:], r2[:, lo:], 1.0 / BASE, CB1,
                        op0=Alu.mult, op1=Alu.add,
                    )
                    m1 = esp.tile([128, PC], f32, name="m1", tag="m1")
                    nc.vector.tensor_scalar(
                        m1[:, lo:], z1[:, lo:], MAGIC, MAGIC,
                        op0=Alu.add, op1=Alu.subtract,
                    )
                    m0 = esp.tile([128, PC], f32, name="m0", tag="m0")
                    nc.vector.scalar_tensor_tensor(
                        m0[:, lo:], in0=m1[:, lo:], scalar=-BASE, in1=r2[:, lo:],
                        op0=Alu.mult, op1=Alu.add,
                    )
                    gs = stg.tile([128, PAD], f32, name="gs", tag="gs")
                    gs3 = gs.rearrange("p (t u) -> p t u", u=3)
                    for u, mu in ((2, m2), (1, m1), (0, m0)):
                        au = esp.tile([128, PC], f32, name=f"a{u}", tag="au")
                        nc.scalar.activation(
                            au[:, lo:], mu[:, lo:], Act.Copy,
                            scale=rs0[:, m : m + 1],
                        )
                        nc.vector.tensor_tensor(
                            gs3[:, lo:, u], au[:, lo:], csj3[:, lo:, u],
                            op=Alu.mult,
                        )
                    nc.sync.dma_start(
                        g_out[
                            m * 128 : (m + 1) * 128,
                            d * RP + 3 * lo : (d + 1) * RP,
                        ],
                        gs[:, 3 * lo : RP],
                    )

    nc.compile()
    return nc


_CACHE = {}


def get_nc(N, D, KN, NCORES):
    key = (N, D, KN, NCORES)
    if key not in _CACHE:
        _CACHE[key] = build_nc(N, D, KN, NCORES)
    return _CACHE[key]


def kernel(feats, kn, _trace=False):
    feats = np.asarray(feats, dtype=np.float32)
    kn = int(kn)
    N, D = feats.shape
    NCORES = 8
    ND = 5
    RP = N // NCORES
    nc = get_nc(N, D, kn, NCORES)
    ident = np.eye(128, dtype=np.float32)
    in_maps = []
    for c in range(NCORES):
        jsel = np.zeros((1, 8), np.int32)
        for d in range(8):
            jsel[0, d] = (c + d) % NCORES
        in_maps.append({
            "feats_all": feats,
            "feats_my": feats[c * RP : (c + 1) * RP],
            "ident_in": ident,
            "jsel_in": jsel,
        })
    res = run_bass_kernel_spmd(
        nc, in_maps, core_ids=list(range(NCORES)), trace=_trace
    )
    out = np.empty((N, N), dtype=np.float32)
    res_g = [res.results[c]["g_out"] for c in range(NCORES)]  # [RP, ND*RP]
    for c in range(NCORES):
        g = res_g[c]
        for d in range(ND):
            j = (c + d) % NCORES
            blk = g[:, d * RP : (d + 1) * RP]
            if d == 0:
                # kernel computed upper triangle only; mirror below diag
                full = np.triu(blk) + np.triu(blk, 1).T
                out[c * RP : (c + 1) * RP, c * RP : (c + 1) * RP] = full
            elif d == ND - 1:
                if c >= NCORES // 2:
                    continue
                # core c has triu of (c,j); core j has triu of (j,c) = tril^T
                blk2 = res_g[j][:, (ND - 1) * RP : ND * RP]
                full = np.triu(blk) + np.tril(blk2.T, -1)
                out[c * RP : (c + 1) * RP, j * RP : (j + 1) * RP] = full
                out[j * RP : (j + 1) * RP, c * RP : (c + 1) * RP] = full.T
            else:
                out[c * RP : (c + 1) * RP, j * RP : (j + 1) * RP] = blk
                out[j * RP : (j + 1) * RP, c * RP : (c + 1) * RP] = blk.T
    if _trace:
        return out, res
    return out


if __name__ == "__main__":
    inputs = {
        "feats": np.load("/tmp/feats.npy"),
        "kn": 10,
    }
    out = kernel(**inputs)
    print("out", out.shape, out.dtype, float(np.abs(out).max()))



# revision 46
# speedup vs baseline: 1.3555x; 1.0048x over previous
"""Trainium2 Bass kernel for nn_Attention_16011638079620 (gnn_message_passing).

Computes, for feats [8192, 256] f32 and kn=10:
    sim   = cosine-similarity(feats)            [N, N]
    B     = rowwise top-kn one-hot mask of softmax(sim) (rank-preserving)
    G     = (1/kn) * invdv_i * invdv_j * (B^T B)_ij,  dv = colsums of B

Strategy (8 cores):
  - sim via 3-pass bf16 hi/lo split matmuls (exact top-k, 4x faster than f32)
  - B columns packed 3-per-fp16 value (base 24; counts <= 23 so the packed
    matmul B^T @ packedB is integer-exact in fp32 PSUM) -> 1.5x fp8-DR rate
  - G is symmetric: core c computes blocks (c, (c+d)%8) for d=0..4; host
    mirrors the rest. lhsT is always the core's own column slice (AllToAll).
  - dv via fp8-DoubleRow ones-matmuls, interleaved with the sim phase.
  - software pipeline: mask/pack of block m-1 runs behind sim of block m so
    PSUM banks release early; collectives merged in block pairs.
"""

import sys

sys.path.insert(0, "/opt/trn_rl_repo")

from contextlib import ExitStack

import numpy as np

import concourse.bass as bass
import concourse.tile as tile
from concourse import bacc, mybir
from concourse.bass import _add_dep_helper
from concourse.bass_utils import run_bass_kernel_spmd

f32 = mybir.dt.float32
bf16 = mybir.dt.bfloat16
fp16 = mybir.dt.float16
fp8 = mybir.dt.float8e4
i32 = mybir.dt.int32
Alu = mybir.AluOpType
Act = mybir.ActivationFunctionType
NEG = -1e30
BASE = 24.0
B2 = BASE * BASE  # 576
MAGIC = 12582912.0  # 1.5 * 2**23: (z + MAGIC) - MAGIC == round-to-nearest(z)
CB2 = 0.5 / B2 - 0.5  # bias so round(P/576 + CB2) == floor(P/576) exactly
CB1 = 0.5 / BASE - 0.5


def build_nc(N, D, KN, NCORES):
    RP = N // NCORES           # 1024 rows/G-rows per core
    MB = RP // 128             # 8 row blocks per core
    NCH = N // 512             # topk chunks
    DT = D // 128              # 2 feature chunks
    PC = 342                   # packed cols per 1024-col slice (342*3 = 1026)
    PCW = NCORES * PC          # 2736 packed cols total per row
    PAD = 3 * PC               # 1026 padded cols per slice
    BW = NCORES * PAD          # 8208 padded mask width
    KK = N // 128              # 64 contraction chunks for phase D
    NP = MB // 2               # 4 block pairs for collectives
    ND = 5                     # symmetric blocks per core
    assert 8 < KN <= 16

    inv_de = float(np.float32(1.0) / np.float32(KN))

    nc = bacc.Bacc(
        "TRN2",
        target_bir_lowering=False,
        debug=False,
        enable_asserts=False,
        num_devices=NCORES,
    )
    feats_all = nc.dram_tensor("feats_all", [N, D], f32, kind="ExternalInput").ap()
    feats_my = nc.dram_tensor("feats_my", [RP, D], f32, kind="ExternalInput").ap()
    ident_in = nc.dram_tensor("ident_in", [128, 128], f32, kind="ExternalInput").ap()
    jsel_in = nc.dram_tensor("jsel_in", [1, 8], i32, kind="ExternalInput").ap()
    g_out = nc.dram_tensor("g_out", [RP, ND * RP], f32, kind="ExternalOutput").ap()

    rg = [list(range(NCORES))]

    with tile.TileContext(nc) as tc, ExitStack() as ctx:
        dram = ctx.enter_context(tc.tile_pool(name="dram", bufs=1, space="DRAM"))
        b_grp2 = [
            dram.tile([NCORES, 2, 128, RP], fp8, name=f"b_grp2_{p}") for p in range(NP)
        ]
        lts_d2 = [
            dram.tile([NCORES, 2, 128, RP], fp8, name=f"lts_d2_{p}") for p in range(NP)
        ]
        # pk exchange is an AllGather: one cheap contiguous write per block
        # (A2A variants need 5x duplicated payload staging, which drowns the
        # sync engine in DIRECT2D descriptor work and stalls the sim phase)
        pk_in2 = [dram.tile([2, 128, PCW], fp16, name=f"pk_in2_{p}") for p in range(NP)]
        pk_ag2 = [
            dram.tile(
                [NCORES, 2, 128, PCW], fp16, addr_space="Shared", name=f"pk_ag2_{p}"
            )
            for p in range(NP)
        ]
        dv_my_d = dram.tile([RP], f32, name="dv_my_d")
        dv_full = dram.tile([N], f32, addr_space="Shared", name="dv_full")
        cs_dram = dram.tile([N], f32, name="cs_dram")

        pers = ctx.enter_context(tc.tile_pool(name="pers", bufs=1))
        dv_ps = None  # allocated from g_ps during the G d0 window

        ident = pers.tile([128, 128], f32, name="ident")
        nc.sync.dma_start(ident[:], ident_in)
        idb = pers.tile([128, 128], bf16, name="idb")
        nc.vector.tensor_copy(idb[:], ident[:])
        jsel_sb = pers.tile([1, 8], i32, name="jsel_sb")
        nc.sync.dma_start(jsel_sb[:], jsel_in)
        ones1 = pers.tile([1, 128], f32, name="ones1")
        nc.vector.memset(ones1[:], 1.0)
        zlh = pers.tile([1, 128], fp16, name="zlh")
        nc.vector.memset(zlh[:], 0.0)
        zrh = pers.tile([1, 512], fp16, name="zrh")
        nc.vector.memset(zrh[:], 0.0)
        ones_dr = pers.tile([128, 2, 16], fp8, name="ones_dr")
        nc.vector.memset(ones_dr[:], 1.0)

        # lhsT (own dense column slice) is loaded per pair in phase D

        # ---------------- phase 1: normalize + hi/lo split + transpose ------
        with ExitStack() as p12:
            fsb = p12.enter_context(tc.tile_pool(name="fsb", bufs=1))
            fnt_hi = [fsb.tile([128, N], bf16, name=f"fh{h}") for h in range(DT)]
            fnt_lo = [fsb.tile([128, N], bf16, name=f"fl{h}") for h in range(DT)]
            fnt_myh = [fsb.tile([128, RP], bf16, name=f"fmh{h}") for h in range(DT)]
            fnt_myl = [fsb.tile([128, RP], bf16, name=f"fml{h}") for h in range(DT)]

            with ExitStack() as p1:
                wrk = p1.enter_context(tc.tile_pool(name="wrk", bufs=3))
                sml = p1.enter_context(tc.tile_pool(name="sml", bufs=6))
                tp_ps = p1.enter_context(
                    tc.tile_pool(name="tp_ps", bufs=2, space="PSUM")
                )

                def norm_group(src4, dh, dl, col0, nb):
                    # nb row-blocks batched: one op set for the whole group
                    ft4 = wrk.tile([128, nb, D], f32, name="ft4")
                    nc.sync.dma_start(ft4[:], src4)
                    tps = {}
                    for x in range(2):
                        tps[x] = tp_ps.tile(
                            [128, DT, nb * 128], bf16, name=f"tp{x}", tag=f"tp{x}"
                        )
                    sq4 = wrk.tile([128, nb, D], f32, name="sq4")
                    nc.scalar.square(
                        sq4.rearrange("p b d -> p (b d)"),
                        ft4.rearrange("p b d -> p (b d)"),
                    )
                    n24 = sml.tile([128, nb, 1], f32, name="n24")
                    nc.vector.reduce_sum(n24[:], sq4[:], axis=mybir.AxisListType.X)
                    nrm4 = sml.tile([128, nb, 1], f32, name="nrm4")
                    nc.scalar.sqrt(
                        nrm4.rearrange("p b o -> p (b o)"),
                        n24.rearrange("p b o -> p (b o)"),
                    )
                    inv4 = sml.tile([128, nb, 1], f32, name="inv4")
                    nc.vector.reciprocal(
                        inv4.rearrange("p b o -> p (b o)"),
                        nrm4.rearrange("p b o -> p (b o)"),
                    )
                    fn4 = wrk.tile([128, nb, D], f32, name="fn4")
                    nc.vector.tensor_tensor(
                        fn4[:], ft4[:], inv4[:].broadcast_to([128, nb, D]),
                        op=Alu.mult,
                    )
                    fh4 = wrk.tile([128, nb, D], bf16, name="fh4")
                    nc.scalar.copy(
                        fh4.rearrange("p b d -> p (b d)"),
                        fn4.rearrange("p b d -> p (b d)"),
                    )
                    fl4 = wrk.tile([128, nb, D], bf16, name="fl4")
                    nc.vector.tensor_tensor(
                        fl4[:], fn4[:], fh4[:], op=Alu.subtract
                    )
                    for i in range(nb):
                        for h in range(DT):
                            for x, s4 in ((0, fh4), (1, fl4)):
                                nc.tensor.transpose(
                                    tps[x][:, h, i * 128 : (i + 1) * 128],
                                    s4[:, i, h * 128 : (h + 1) * 128],
                                    idb[:],
                                )
                    for h in range(DT):
                        for x, dst in ((0, dh), (1, dl)):
                            nc.scalar.copy(
                                dst[h][:, col0 : col0 + nb * 128], tps[x][:, h, :]
                            )

                fm4 = feats_my.rearrange("(g i p) d -> g p i d", p=128, i=4)
                for g in range(MB // 4):
                    norm_group(fm4[g], fnt_myh, fnt_myl, g * 512, 4)
                fa4 = feats_all.rearrange("(g i p) d -> g p i d", p=128, i=4)
                for g in range(N // 512):
                    norm_group(fa4[g], fnt_hi, fnt_lo, g * 512, 4)

            # ---------------- phase 2: sim, topk, mask, pack, CC -----------
            with ExitStack() as p2:
                simp = p2.enter_context(tc.tile_pool(name="simp", bufs=3))
                smal = p2.enter_context(tc.tile_pool(name="smal", bufs=2))
                bmpp = p2.enter_context(tc.tile_pool(name="bmpp", bufs=1))
                pkp = p2.enter_context(tc.tile_pool(name="pkp", bufs=1))
                t0p = p2.enter_context(tc.tile_pool(name="t0p", bufs=2))
                sim_ps = p2.enter_context(
                    tc.tile_pool(name="sim_ps", bufs=1, space="PSUM")
                )
                combos = []
                for h in range(DT):
                    combos.append((fnt_myh[h], fnt_hi[h]))
                    combos.append((fnt_myh[h], fnt_lo[h]))
                    combos.append((fnt_myl[h], fnt_hi[h]))

                tkns = {}
                halves = {}

                def sim_block(m):
                    sh0 = simp.tile([128, N // 2], f32, name="sh0", tag="sh")
                    sh1 = simp.tile([128, N // 2], f32, name="sh1", tag="sh")
                    halves[m] = (sh0, sh1)
                    cand = smal.tile([128, 8 * NCH], f32, name="cand", tag="cand")
                    for qr in range(4):
                        pss = [
                            sim_ps.tile([128, 512], f32, name=f"sq{t}", tag=f"sq{t}")
                            for t in range(4)
                        ]
                        for ci, (la, ra) in enumerate(combos):
                            lt = la[:, m * 128 : (m + 1) * 128]
                            for t in range(4):
                                ntc = qr * 4 + t
                                nc.tensor.matmul(
                                    pss[t][:],
                                    lt,
                                    ra[:, ntc * 512 : (ntc + 1) * 512],
                                    start=(ci == 0),
                                    stop=(ci == 5),
                                )
                        sh = (sh0, sh1)[qr // 2]
                        for t in range(4):
                            ntc = qr * 4 + t
                            nc.vector.max(
                                cand[:, ntc * 8 : (ntc + 1) * 8], pss[t][:]
                            )
                            nc.scalar.copy(
                                sh[:, (ntc % 8) * 512 : (ntc % 8 + 1) * 512],
                                pss[t][:],
                            )
                    c8 = smal.tile([128, 8], f32, name="c8", tag="c8")
                    nc.vector.max(c8[:], cand[:])
                    cand2 = smal.tile([128, 8 * NCH], f32, name="cand2", tag="cand2")
                    nc.vector.match_replace(cand2[:], c8[:], cand[:], NEG)
                    c8b = smal.tile([128, 8], f32, name="c8b", tag="c8b")
                    nc.vector.max(c8b[:], cand2[:])
                    tkns[m] = c8b

                def mask_pack(m):
                    tkn = tkns[m][:, KN - 9 : KN - 8]
                    sh0, sh1 = halves[m]
                    bmp = bmpp.tile([128, BW], fp8, name="bmp")
                    for j in range(NCORES):
                        sh = (sh0, sh1)[j // 4]
                        nc.vector.tensor_scalar(
                            bmp[:, j * PAD : j * PAD + RP],
                            sh[:, (j % 4) * RP : (j % 4 + 1) * RP],
                            tkn,
                            None,
                            op0=Alu.is_ge,
                        )
                        nc.vector.memset(bmp[:, j * PAD + RP : (j + 1) * PAD], 0.0)
                    pk = pkp.tile([128, PCW], fp16, name="pk")
                    bm3 = bmp.rearrange("p (j t u) -> p j t u", j=NCORES, u=3)
                    for j in range(NCORES):
                        t0 = t0p.tile([128, PC], f32, name="t0")
                        nc.vector.scalar_tensor_tensor(
                            t0[:], in0=bm3[:, j, :, 1], scalar=BASE,
                            in1=bm3[:, j, :, 0], op0=Alu.mult, op1=Alu.add,
                        )
                        nc.vector.scalar_tensor_tensor(
                            pk[:, j * PC : (j + 1) * PC],
                            in0=bm3[:, j, :, 2], scalar=B2, in1=t0[:],
                            op0=Alu.mult, op1=Alu.add,
                        )
                    pp, i = m // 2, m % 2
                    nc.sync.dma_start(pk_in2[pp][i], pk[:])
                    nc.sync.dma_start(
                        b_grp2[pp][:, i].rearrange("j p q -> p j q"),
                        bmp.rearrange("p (j q) -> p j q", j=NCORES)[:, :, 0:RP],
                    )

                def cc_pk5(pp):
                    nc.gpsimd.collective_compute(
                        "AllGather", Alu.bypass, replica_groups=rg,
                        ins=[pk_in2[pp].opt()], outs=[pk_ag2[pp].opt()],
                    )

                def cc_pair(pp, defer_pk=False):
                    # small lt A2A first so dv/G lhs unblocks earliest
                    nc.gpsimd.collective_compute(
                        "AllToAll", Alu.bypass, replica_groups=rg,
                        ins=[b_grp2[pp].opt()], outs=[lts_d2[pp].opt()],
                    )
                    if not defer_pk:
                        cc_pk5(pp)

                for m in range(MB):
                    sim_block(m)
                    if m >= 1:
                        mask_pack(m - 1)
                        if (m - 1) % 2 == 1:
                            cc_pair((m - 1) // 2)
                mask_pack(MB - 1)
                # defer pair-3 pk AG so the dv AllGather can jump the CC queue
                cc_pair(NP - 1, defer_pk=True)
                cc_pk5_fn = cc_pk5

        # ---------------- phase 3+4: pair-major G, SBUF accumulation --------
        # G units (pair, d, m) run as each pair's collectives land, so PE
        # work rides inside the CC windows; per-(d,m) psum partials are
        # added into SBUF accumulators and only pair-3's sweep + the output
        # stage remain after the last arrival.
        G0 = [(m * 128) // 3 for m in range(MB)]
        with ExitStack() as p4:
            gw = p4.enter_context(tc.tile_pool(name="gw", bufs=4))
            csp = p4.enter_context(tc.tile_pool(name="csp", bufs=2))
            stg = p4.enter_context(tc.tile_pool(name="stg", bufs=2))
            esp = p4.enter_context(tc.tile_pool(name="esp", bufs=1))
            accp = p4.enter_context(tc.tile_pool(name="accp", bufs=1))
            rhp = p4.enter_context(tc.tile_pool(name="rhp", bufs=2))
            g_ps = p4.enter_context(tc.tile_pool(name="g_ps", bufs=1, space="PSUM"))

            acc = {
                d: accp.tile([128, MB, PC], f32, name=f"acc{d}")
                for d in range(ND)
            }

            def los_of(d):
                return G0 if d in (0, ND - 1) else [0] * MB

            # lhsT ring: each pair's dense column slice is only used by
            # that pair's units + dv, so a 2-deep ring replaces the 8MB
            # persistent lt_all
            ltrp = p4.enter_context(tc.tile_pool(name="ltrp", bufs=2))

            jds = {}
            for dd in range(ND):
                jr = nc.sync.alloc_register(f"jr{dd}")
                nc.sync.reg_load(jr, jsel_sb[0:1, dd : dd + 1])
                jds[dd] = nc.sync.snap(
                    jr, donate=True, min_val=0, max_val=NCORES - 1
                )

            def load_rh5(pp, d):
                rh = rhp.tile([128, 2, NCORES, PC], fp16, name="rh5", tag="rh5")
                for i in range(2):
                    nc.sync.dma_start(
                        rh[:, i],
                        pk_ag2[pp][:, i, :, :].rearrange("o p c -> p o c")[
                            :, :, bass.ds(jds[d] * PC, PC)
                        ],
                    )
                return rh

            dv_ps = g_ps.tile([128, 512], f32, name="dv_ps", tag="gp7")
            uc = 0
            csts = {}
            rs0 = None
            csjs = {}

            def scales_block():
                # rowscale: rs = invdv(my col block) * inv_de
                nonlocal rs0
                d1 = gw.tile([128, MB], f32, name="d1")
                nc.vector.tensor_scalar_max(d1[:], dv_sb[:], 1.0)
                sq = gw.tile([128, MB], f32, name="sqv")
                nc.scalar.sqrt(sq[:], d1[:])
                rc = gw.tile([128, MB], f32, name="rc")
                nc.vector.reciprocal(rc[:], sq[:])
                mk = gw.tile([128, MB], f32, name="mk")
                nc.vector.tensor_scalar(mk[:], dv_sb[:], 0.0, None, op0=Alu.is_gt)
                iv = gw.tile([128, MB], f32, name="iv")
                nc.vector.tensor_tensor(iv[:], rc[:], mk[:], op=Alu.mult)
                rs0 = gw.tile([128, MB], f32, name="rsu0")
                nc.vector.tensor_scalar_mul(rs0[:], iv[:], inv_de)

                # colscale source: cs = invdv over all N (from dv AllGather)
                q = N // 128
                dvw = gw.tile([128, q], f32, name="dvw")
                nc.scalar.dma_start(
                    dvw[:], dv_full.rearrange("(cm p) -> p cm", p=128)
                )
                d1w = gw.tile([128, q], f32, name="d1w")
                nc.vector.tensor_scalar_max(d1w[:], dvw[:], 1.0)
                sqw = gw.tile([128, q], f32, name="sqw")
                nc.scalar.sqrt(sqw[:], d1w[:])
                rcw = gw.tile([128, q], f32, name="rcw")
                nc.vector.reciprocal(rcw[:], sqw[:])
                mkw = gw.tile([128, q], f32, name="mkw")
                nc.vector.tensor_scalar(mkw[:], dvw[:], 0.0, None, op0=Alu.is_gt)
                ivw = gw.tile([128, q], f32, name="ivw")
                nc.vector.tensor_tensor(ivw[:], rcw[:], mkw[:], op=Alu.mult)
                nc.scalar.dma_start(
                    cs_dram.rearrange("(cm p) -> p cm", p=128), ivw[:]
                )
                cs2d = cs_dram.rearrange("(a n) -> a n", a=1)
                for dd in range(ND):
                    jra = nc.scalar.alloc_register(f"jra{dd}")
                    nc.scalar.reg_load(jra, jsel_sb[0:1, dd : dd + 1])
                    jda = nc.scalar.snap(
                        jra, donate=True, min_val=0, max_val=NCORES - 1
                    )
                    cs_t = gw.tile([1, RP], f32, name="cs_t", tag=f"cs_t{dd % 2}")
                    nc.scalar.dma_start(cs_t[:], cs2d[:, bass.ds(jda * RP, RP)])
                    csts[dd] = cs_t

            def output_stage(d):
                nonlocal uc
                los = los_of(d)
                csj = csp.tile([128, PAD], f32, name="csj", tag="csj")
                for hh in range(2):
                    cps = g_ps.tile([128, 512], f32, name="cps", tag=f"gp{uc % 7}")
                    uc += 1
                    nc.tensor.matmul(
                        cps[:], ones1[:],
                        csts[d][:, hh * 512 : (hh + 1) * 512],
                        start=True, stop=True,
                    )
                    nc.scalar.copy(csj[:, hh * 512 : (hh + 1) * 512], cps[:])
                nc.vector.memset(csj[:, RP:PAD], 0.0)
                csj3 = csj.rearrange("p (t u) -> p t u", u=3)
                for m in range(MB):
                    lo = los[m]
                    es = acc[d][:, m, :]
                    z2 = esp.tile([128, PC], f32, name="z2", tag="z2")
                    nc.vector.tensor_scalar(
                        z2[:, lo:], es[:, lo:], 1.0 / B2, CB2,
                        op0=Alu.mult, op1=Alu.add,
                    )
                    m2 = esp.tile([128, PC], f32, name="m2", tag="m2")
                    nc.vector.tensor_scalar(
                        m2[:, lo:], z2[:, lo:], MAGIC, MAGIC,
                        op0=Alu.add, op1=Alu.subtract,
                    )
                    r2 = esp.tile([128, PC], f32, name="r2", tag="r2")
                    nc.vector.scalar_tensor_tensor(
                        r2[:, lo:], in0=m2[:, lo:], scalar=-B2, in1=es[:, lo:],
                        op0=Alu.mult, op1=Alu.add,
                    )
                    z1 = esp.tile([128, PC], f32, name="z1", tag="z1")
                    nc.vector.tensor_scalar(
                        z1[:, lo:], r2[:, lo:], 1.0 / BASE, CB1,
                        op0=Alu.mult, op1=Alu.add,
                    )
                    m1 = esp.tile([128, PC], f32, name="m1", tag="m1")
                    nc.vector.tensor_scalar(
                        m1[:, lo:], z1[:, lo:], MAGIC, MAGIC,
                        op0=Alu.add, op1=Alu.subtract,
                    )
                    m0 = esp.tile([128, PC], f32, name="m0", tag="m0")
                    nc.vector.scalar_tensor_tensor(
                        m0[:, lo:], in0=m1[:, lo:], scalar=-BASE, in1=r2[:, lo:],
                        op0=Alu.mult, op1=Alu.add,
                    )
                    gs = stg.tile([128, PAD], f32, name="gs", tag="gs")
                    gs3 = gs.rearrange("p (t u) -> p t u", u=3)
                    for u, mu in ((2, m2), (1, m1), (0, m0)):
                        au = esp.tile([128, PC], f32, name=f"a{u}", tag="au")
                        nc.scalar.activation(
                            au[:, lo:], mu[:, lo:], Act.Copy,
                            scale=rs0[:, m : m + 1],
                        )
                        nc.vector.tensor_tensor(
                            gs3[:, lo:, u], au[:, lo:], csj3[:, lo:, u],
                            op=Alu.mult,
                        )
                    nc.sync.dma_start(
                        g_out[
                            m * 128 : (m + 1) * 128,
                            d * RP + 3 * lo : (d + 1) * RP,
                        ],
                        gs[:, 3 * lo : RP],
                    )

            for pp in range(NP):
                ltr = ltrp.tile([128, 2, NCORES, RP], fp8, name="ltr", tag="ltr")
                for ii in range(2):
                    nc.sync.dma_start(
                        ltr[:, ii],
                        lts_d2[pp][:, ii].rearrange("o p q -> p o q"),
                    )
                # dv first: pair-3's dv only needs lt3, so the dv AllGather
                # can run ahead of the pair-3 pk AllGather in the CC stream
                for o in range(NCORES):
                    for m in range(MB):
                        nc.tensor.matmul(
                            dv_ps[:, m : m + 1],
                            ltr[:, :, o, m * 128 : (m + 1) * 128],
                            ones_dr[:, :, 0:1],
                            perf_mode=mybir.MatmulPerfMode.DoubleRow,
                            start=(pp == 0 and o == 0 and m == 0),
                            stop=(
                                pp == NP - 1 and o == NCORES - 1 and m == MB - 1
                            ),
                            skip_group_check=True,
                        )
                if pp == NP - 1:
                    dv_sb = gw.tile([128, MB], f32, name="dv_sb")
                    nc.vector.tensor_copy(dv_sb[:], dv_ps[:, 0:MB])
                    nc.scalar.dma_start(
                        dv_my_d.rearrange("(m p) -> p m", p=128), dv_sb[:]
                    )
                    nc.gpsimd.collective_compute(
                        "AllGather", Alu.bypass, replica_groups=rg,
                        ins=[dv_my_d.opt()], outs=[dv_full.opt()],
                    )
                    cc_pk5_fn(NP - 1)
                    scales_block()
                rhs = {}
                rhs[0] = load_rh5(pp, 0)
                rhs[1] = load_rh5(pp, 1)
                rhs[2] = load_rh5(pp, 2)
                for d in range(ND):
                    if d + 3 < ND:
                        rhs[d + 3] = load_rh5(pp, d + 3)
                    los = los_of(d)
                    rh = rhs.pop(d)
                    for m in range(MB):
                        lo = los[m]
                        ps = g_ps.tile(
                            [128, 512], f32, name="gps", tag=f"gp{uc % 7}"
                        )
                        uc += 1
                        k = 0
                        for i in range(2):
                            for o in range(NCORES):
                                nc.tensor.matmul(
                                    ps[:, 0 : PC - lo],
                                    ltr[:, i, o, m * 128 : (m + 1) * 128],
                                    rh[:, i, o, lo:PC],
                                    start=(k == 0),
                                    stop=(k == 15),
                                    skip_group_check=True,
                                )
                                k += 1
                        asl = acc[d][:, m, lo:PC]
                        if pp == 0:
                            nc.scalar.copy(asl, ps[:, 0 : PC - lo])
                        else:
                            nc.vector.tensor_tensor(
                                asl, asl, ps[:, 0 : PC - lo], op=Alu.add
                            )
                    if pp == NP - 1:
                        # acc[d] is final: emit its output stage now so the
                        # unpack/scale work overlaps the remaining units
                        output_stage(d)

    nc.compile()
    return nc


_CACHE = {}


def get_nc(N, D, KN, NCORES):
    key = (N, D, KN, NCORES)
    if key not in _CACHE:
        _CACHE[key] = build_nc(N, D, KN, NCORES)
    return _CACHE[key]


def kernel(feats, kn, _trace=False):
    feats = np.asarray(feats, dtype=np.float32)
    kn = int(kn)
    N, D = feats.shape
    NCORES = 8
    ND = 5
    RP = N // NCORES
    nc = get_nc(N, D, kn, NCORES)
    ident = np.eye(128, dtype=np.float32)
    in_maps = []
    for c in range(NCORES):
        jsel = np.zeros((1, 8), np.int32)
        for d in range(8):
            jsel[0, d] = (c + d) % NCORES
        in_maps.append({
            "feats_all": feats,
            "feats_my": feats[c * RP : (c + 1) * RP],
            "ident_in": ident,
            "jsel_in": jsel,
        })
    res = run_bass_kernel_spmd(
        nc, in_maps, core_ids=list(range(NCORES)), trace=_trace
    )
    out = np.empty((N, N), dtype=np.float32)
    res_g = [res.results[c]["g_out"] for c in range(NCORES)]  # [RP, ND*RP]
    for c in range(NCORES):
        g = res_g[c]
        for d in range(ND):
            j = (c + d) % NCORES
            blk = g[:, d * RP : (d + 1) * RP]
            if d == 0:
                # kernel computed upper triangle only; mirror below diag
                full = np.triu(blk) + np.triu(blk, 1).T
                out[c * RP : (c + 1) * RP, c * RP : (c + 1) * RP] = full
            elif d == ND - 1:
                if c >= NCORES // 2:
                    continue
                # core c has triu of (c,j); core j has triu of (j,c) = tril^T
                blk2 = res_g[j][:, (ND - 1) * RP : ND * RP]
                full = np.triu(blk) + np.tril(blk2.T, -1)
                out[c * RP : (c + 1) * RP, j * RP : (j + 1) * RP] = full
                out[j * RP : (j + 1) * RP, c * RP : (c + 1) * RP] = full.T
            else:
                out[c * RP : (c + 1) * RP, j * RP : (j + 1) * RP] = blk
                out[j * RP : (j + 1) * RP, c * RP : (c + 1) * RP] = blk.T
    if _trace:
        return out, res
    return out


if __name__ == "__main__":
    inputs = {
        "feats": np.load("/tmp/feats.npy"),
        "kn": 10,
    }
    out = kernel(**inputs)
    print("out", out.shape, out.dtype, float(np.abs(out).max()))



# revision 51
# speedup vs baseline: 1.4056x; 1.0370x over previous
"""Trainium2 Bass kernel for nn_Attention_16011638079620 (gnn_message_passing).

Computes, for feats [8192, 256] f32 and kn=10:
    sim   = cosine-similarity(feats)            [N, N]
    B     = rowwise top-kn one-hot mask of softmax(sim) (rank-preserving)
    G     = (1/kn) * invdv_i * invdv_j * (B^T B)_ij,  dv = colsums of B

Strategy (8 cores):
  - sim via 3-pass bf16 hi/lo split matmuls (exact top-k, 4x faster than f32)
  - B columns packed 3-per-fp16 value (base 24; counts <= 23 so the packed
    matmul B^T @ packedB is integer-exact in fp32 PSUM) -> 1.5x fp8-DR rate
  - G is symmetric: core c computes blocks (c, (c+d)%8) for d=0..4; host
    mirrors the rest. lhsT is always the core's own column slice (AllToAll).
  - dv via fp8-DoubleRow ones-matmuls, interleaved with the sim phase.
  - software pipeline: mask/pack of block m-1 runs behind sim of block m so
    PSUM banks release early; collectives merged in block pairs.
"""

import sys

sys.path.insert(0, "/opt/trn_rl_repo")

from contextlib import ExitStack

import numpy as np

import concourse.bass as bass
import concourse.tile as tile
from concourse import bacc, mybir
from concourse.bass import _add_dep_helper
from concourse.bass_utils import run_bass_kernel_spmd

f32 = mybir.dt.float32
bf16 = mybir.dt.bfloat16
fp16 = mybir.dt.float16
fp8 = mybir.dt.float8e4
i32 = mybir.dt.int32
Alu = mybir.AluOpType
Act = mybir.ActivationFunctionType
NEG = -1e30
BASE = 24.0
B2 = BASE * BASE  # 576
MAGIC = 12582912.0  # 1.5 * 2**23: (z + MAGIC) - MAGIC == round-to-nearest(z)
CB2 = 0.5 / B2 - 0.5  # bias so round(P/576 + CB2) == floor(P/576) exactly
CB1 = 0.5 / BASE - 0.5


def build_nc(N, D, KN, NCORES):
    RP = N // NCORES           # 1024 rows/G-rows per core
    MB = RP // 128             # 8 row blocks per core
    NCH = N // 512             # topk chunks
    DT = D // 128              # 2 feature chunks
    PC = 342                   # packed cols per 1024-col slice (342*3 = 1026)
    PCW = NCORES * PC          # 2736 packed cols total per row
    PAD = 3 * PC               # 1026 padded cols per slice
    BW = NCORES * PAD          # 8208 padded mask width
    KK = N // 128              # 64 contraction chunks for phase D
    NP = MB // 2               # 4 block pairs for collectives
    ND = 5                     # symmetric blocks per core
    assert 8 < KN <= 16

    inv_de = float(np.float32(1.0) / np.float32(KN))

    nc = bacc.Bacc(
        "TRN2",
        target_bir_lowering=False,
        debug=False,
        enable_asserts=False,
        num_devices=NCORES,
    )
    feats_all = nc.dram_tensor("feats_all", [N, D], f32, kind="ExternalInput").ap()
    feats_my = nc.dram_tensor("feats_my", [RP, D], f32, kind="ExternalInput").ap()
    ident_in = nc.dram_tensor("ident_in", [128, 128], f32, kind="ExternalInput").ap()
    jsel_in = nc.dram_tensor("jsel_in", [1, 8], i32, kind="ExternalInput").ap()
    g_out = nc.dram_tensor("g_out", [RP, ND * RP], f32, kind="ExternalOutput").ap()

    rg = [list(range(NCORES))]

    with tile.TileContext(nc) as tc, ExitStack() as ctx:
        dram = ctx.enter_context(tc.tile_pool(name="dram", bufs=1, space="DRAM"))
        b_grp2 = [
            dram.tile([NCORES, 2, 128, RP], fp8, name=f"b_grp2_{p}") for p in range(NP)
        ]
        lts_d2 = [
            dram.tile([NCORES, 2, 128, RP], fp8, name=f"lts_d2_{p}") for p in range(NP)
        ]
        # pk exchange is an AllGather: one cheap contiguous write per block
        # (A2A variants need 5x duplicated payload staging, which drowns the
        # sync engine in DIRECT2D descriptor work and stalls the sim phase)
        pk_in2 = [dram.tile([2, 128, PCW], fp16, name=f"pk_in2_{p}") for p in range(NP)]
        pk_ag2 = [
            dram.tile(
                [NCORES, 2, 128, PCW], fp16, addr_space="Shared", name=f"pk_ag2_{p}"
            )
            for p in range(NP)
        ]
        dv_my_d = dram.tile([RP], f32, name="dv_my_d")
        dv_full = dram.tile([N], f32, addr_space="Shared", name="dv_full")
        cs_dram = dram.tile([N], f32, name="cs_dram")

        pers = ctx.enter_context(tc.tile_pool(name="pers", bufs=1))
        dv_ps = None  # allocated from g_ps during the G d0 window

        ident = pers.tile([128, 128], f32, name="ident")
        nc.sync.dma_start(ident[:], ident_in)
        idb = pers.tile([128, 128], bf16, name="idb")
        nc.vector.tensor_copy(idb[:], ident[:])
        jsel_sb = pers.tile([1, 8], i32, name="jsel_sb")
        nc.sync.dma_start(jsel_sb[:], jsel_in)
        ones1 = pers.tile([1, 128], f32, name="ones1")
        nc.vector.memset(ones1[:], 1.0)
        zlh = pers.tile([1, 128], fp16, name="zlh")
        nc.vector.memset(zlh[:], 0.0)
        zrh = pers.tile([1, 512], fp16, name="zrh")
        nc.vector.memset(zrh[:], 0.0)
        ones_dr = pers.tile([128, 2, 16], fp8, name="ones_dr")
        nc.vector.memset(ones_dr[:], 1.0)

        # lhsT (own dense column slice) is loaded per pair in phase D.
        # These pools live at ctx scope so pair-0's loads can be issued
        # from inside the sim loop (ahead of mask_pack(7)'s DMA writes on
        # the in-order sync queue).
        rhp = ctx.enter_context(tc.tile_pool(name="rhp", bufs=3))
        ltrp = ctx.enter_context(tc.tile_pool(name="ltrp", bufs=2))
        jds = {}
        for dd in range(ND):
            jr = nc.sync.alloc_register(f"jr{dd}")
            nc.sync.reg_load(jr, jsel_sb[0:1, dd : dd + 1])
            jds[dd] = nc.sync.snap(jr, donate=True, min_val=0, max_val=NCORES - 1)

        def load_rh5(pp, d):
            rh = rhp.tile([128, 2, NCORES, PC], fp16, name="rh5", tag="rh5")
            for i in range(2):
                nc.sync.dma_start(
                    rh[:, i],
                    pk_ag2[pp][:, i, :, :].rearrange("o p c -> p o c")[
                        :, :, bass.ds(jds[d] * PC, PC)
                    ],
                )
            return rh

        def load_ltr(pp):
            ltr = ltrp.tile([128, 2, NCORES, RP], fp8, name="ltr", tag="ltr")
            for ii in range(2):
                nc.sync.dma_start(
                    ltr[:, ii],
                    lts_d2[pp][:, ii].rearrange("o p q -> p o q"),
                )
            return ltr

        pre0 = {}

        # ---------------- phase 1: normalize + hi/lo split + transpose ------
        with ExitStack() as p12:
            fsb = p12.enter_context(tc.tile_pool(name="fsb", bufs=1))
            fnt_hi = [fsb.tile([128, N], bf16, name=f"fh{h}") for h in range(DT)]
            fnt_lo = [fsb.tile([128, N], bf16, name=f"fl{h}") for h in range(DT)]
            fnt_myh = [fsb.tile([128, RP], bf16, name=f"fmh{h}") for h in range(DT)]
            fnt_myl = [fsb.tile([128, RP], bf16, name=f"fml{h}") for h in range(DT)]

            with ExitStack() as p1:
                wrk = p1.enter_context(tc.tile_pool(name="wrk", bufs=3))
                sml = p1.enter_context(tc.tile_pool(name="sml", bufs=6))
                tp_ps = p1.enter_context(
                    tc.tile_pool(name="tp_ps", bufs=2, space="PSUM")
                )

                def norm_group(src4, dh, dl, col0, nb):
                    # nb row-blocks batched: one op set for the whole group
                    ft4 = wrk.tile([128, nb, D], f32, name="ft4")
                    nc.sync.dma_start(ft4[:], src4)
                    tps = {}
                    for x in range(2):
                        tps[x] = tp_ps.tile(
                            [128, DT, nb * 128], bf16, name=f"tp{x}", tag=f"tp{x}"
                        )
                    sq4 = wrk.tile([128, nb, D], f32, name="sq4")
                    nc.scalar.square(
                        sq4.rearrange("p b d -> p (b d)"),
                        ft4.rearrange("p b d -> p (b d)"),
                    )
                    n24 = sml.tile([128, nb, 1], f32, name="n24")
                    nc.vector.reduce_sum(n24[:], sq4[:], axis=mybir.AxisListType.X)
                    nrm4 = sml.tile([128, nb, 1], f32, name="nrm4")
                    nc.scalar.sqrt(
                        nrm4.rearrange("p b o -> p (b o)"),
                        n24.rearrange("p b o -> p (b o)"),
                    )
                    inv4 = sml.tile([128, nb, 1], f32, name="inv4")
                    nc.vector.reciprocal(
                        inv4.rearrange("p b o -> p (b o)"),
                        nrm4.rearrange("p b o -> p (b o)"),
                    )
                    fn4 = wrk.tile([128, nb, D], f32, name="fn4")
                    nc.vector.tensor_tensor(
                        fn4[:], ft4[:], inv4[:].broadcast_to([128, nb, D]),
                        op=Alu.mult,
                    )
                    fh4 = wrk.tile([128, nb, D], bf16, name="fh4")
                    nc.scalar.copy(
                        fh4.rearrange("p b d -> p (b d)"),
                        fn4.rearrange("p b d -> p (b d)"),
                    )
                    fl4 = wrk.tile([128, nb, D], bf16, name="fl4")
                    nc.vector.tensor_tensor(
                        fl4[:], fn4[:], fh4[:], op=Alu.subtract
                    )
                    for i in range(nb):
                        for h in range(DT):
                            for x, s4 in ((0, fh4), (1, fl4)):
                                nc.tensor.transpose(
                                    tps[x][:, h, i * 128 : (i + 1) * 128],
                                    s4[:, i, h * 128 : (h + 1) * 128],
                                    idb[:],
                                )
                    for h in range(DT):
                        for x, dst in ((0, dh), (1, dl)):
                            nc.scalar.copy(
                                dst[h][:, col0 : col0 + nb * 128], tps[x][:, h, :]
                            )

                fm4 = feats_my.rearrange("(g i p) d -> g p i d", p=128, i=4)
                for g in range(MB // 4):
                    norm_group(fm4[g], fnt_myh, fnt_myl, g * 512, 4)
                fa4 = feats_all.rearrange("(g i p) d -> g p i d", p=128, i=4)
                for g in range(N // 512):
                    norm_group(fa4[g], fnt_hi, fnt_lo, g * 512, 4)

            # ---------------- phase 2: sim, topk, mask, pack, CC -----------
            with ExitStack() as p2:
                simp = p2.enter_context(tc.tile_pool(name="simp", bufs=3))
                smal = p2.enter_context(tc.tile_pool(name="smal", bufs=2))
                bmpp = p2.enter_context(tc.tile_pool(name="bmpp", bufs=1))
                pkp = p2.enter_context(tc.tile_pool(name="pkp", bufs=1))
                t0p = p2.enter_context(tc.tile_pool(name="t0p", bufs=2))
                sim_ps = p2.enter_context(
                    tc.tile_pool(name="sim_ps", bufs=1, space="PSUM")
                )
                combos = []
                for h in range(DT):
                    combos.append((fnt_myh[h], fnt_hi[h]))
                    combos.append((fnt_myh[h], fnt_lo[h]))
                    combos.append((fnt_myl[h], fnt_hi[h]))

                tkns = {}
                halves = {}

                def sim_block(m):
                    sh0 = simp.tile([128, N // 2], f32, name="sh0", tag="sh")
                    sh1 = simp.tile([128, N // 2], f32, name="sh1", tag="sh")
                    halves[m] = (sh0, sh1)
                    cand = smal.tile([128, 8 * NCH], f32, name="cand", tag="cand")
                    for qr in range(4):
                        pss = [
                            sim_ps.tile([128, 512], f32, name=f"sq{t}", tag=f"sq{t}")
                            for t in range(4)
                        ]
                        for ci, (la, ra) in enumerate(combos):
                            lt = la[:, m * 128 : (m + 1) * 128]
                            for t in range(4):
                                ntc = qr * 4 + t
                                nc.tensor.matmul(
                                    pss[t][:],
                                    lt,
                                    ra[:, ntc * 512 : (ntc + 1) * 512],
                                    start=(ci == 0),
                                    stop=(ci == 5),
                                )
                        sh = (sh0, sh1)[qr // 2]
                        for t in range(4):
                            ntc = qr * 4 + t
                            nc.vector.max(
                                cand[:, ntc * 8 : (ntc + 1) * 8], pss[t][:]
                            )
                            nc.scalar.copy(
                                sh[:, (ntc % 8) * 512 : (ntc % 8 + 1) * 512],
                                pss[t][:],
                            )
                    c8 = smal.tile([128, 8], f32, name="c8", tag="c8")
                    nc.vector.max(c8[:], cand[:])
                    cand2 = smal.tile([128, 8 * NCH], f32, name="cand2", tag="cand2")
                    nc.vector.match_replace(cand2[:], c8[:], cand[:], NEG)
                    c8b = smal.tile([128, 8], f32, name="c8b", tag="c8b")
                    nc.vector.max(c8b[:], cand2[:])
                    tkns[m] = c8b

                def mask_pack(m):
                    tkn = tkns[m][:, KN - 9 : KN - 8]
                    sh0, sh1 = halves[m]
                    bmp = bmpp.tile([128, BW], fp8, name="bmp")
                    for j in range(NCORES):
                        sh = (sh0, sh1)[j // 4]
                        nc.vector.tensor_scalar(
                            bmp[:, j * PAD : j * PAD + RP],
                            sh[:, (j % 4) * RP : (j % 4 + 1) * RP],
                            tkn,
                            None,
                            op0=Alu.is_ge,
                        )
                        nc.vector.memset(bmp[:, j * PAD + RP : (j + 1) * PAD], 0.0)
                    pk = pkp.tile([128, PCW], fp16, name="pk")
                    bm3 = bmp.rearrange("p (j t u) -> p j t u", j=NCORES, u=3)
                    for j in range(NCORES):
                        t0 = t0p.tile([128, PC], f32, name="t0")
                        nc.vector.scalar_tensor_tensor(
                            t0[:], in0=bm3[:, j, :, 1], scalar=BASE,
                            in1=bm3[:, j, :, 0], op0=Alu.mult, op1=Alu.add,
                        )
                        nc.vector.scalar_tensor_tensor(
                            pk[:, j * PC : (j + 1) * PC],
                            in0=bm3[:, j, :, 2], scalar=B2, in1=t0[:],
                            op0=Alu.mult, op1=Alu.add,
                        )
                    pp, i = m // 2, m % 2
                    nc.sync.dma_start(pk_in2[pp][i], pk[:])
                    nc.sync.dma_start(
                        b_grp2[pp][:, i].rearrange("j p q -> p j q"),
                        bmp.rearrange("p (j q) -> p j q", j=NCORES)[:, :, 0:RP],
                    )

                def cc_pk5(pp):
                    nc.gpsimd.collective_compute(
                        "AllGather", Alu.bypass, replica_groups=rg,
                        ins=[pk_in2[pp].opt()], outs=[pk_ag2[pp].opt()],
                    )

                def cc_pair(pp, defer_pk=False):
                    # small lt A2A first so dv/G lhs unblocks earliest
                    nc.gpsimd.collective_compute(
                        "AllToAll", Alu.bypass, replica_groups=rg,
                        ins=[b_grp2[pp].opt()], outs=[lts_d2[pp].opt()],
                    )
                    if not defer_pk:
                        cc_pk5(pp)

                for m in range(MB):
                    sim_block(m)
                    if m >= 1:
                        mask_pack(m - 1)
                        if (m - 1) % 2 == 1:
                            cc_pair((m - 1) // 2)
                # prefetch pair-0's G operands ahead of mask_pack(7)'s
                # writes (pair-3's AG is deferred, so the small delay to
                # pk_in2[3]/b_grp2[3] is harmless)
                pre0["ltr"] = load_ltr(0)
                pre0[0] = load_rh5(0, 0)
                pre0[1] = load_rh5(0, 1)
                mask_pack(MB - 1)
                # defer pair-3 pk AG so the dv AllGather can jump the CC queue
                cc_pair(NP - 1, defer_pk=True)
                cc_pk5_fn = cc_pk5

        # ---------------- phase 3+4: pair-major G, SBUF accumulation --------
        # G units (pair, d, m) run as each pair's collectives land, so PE
        # work rides inside the CC windows; per-(d,m) psum partials are
        # added into SBUF accumulators and only pair-3's sweep + the output
        # stage remain after the last arrival.
        G0 = [(m * 128) // 3 for m in range(MB)]
        with ExitStack() as p4:
            gw = p4.enter_context(tc.tile_pool(name="gw", bufs=4))
            csp = p4.enter_context(tc.tile_pool(name="csp", bufs=2))
            stg = p4.enter_context(tc.tile_pool(name="stg", bufs=2))
            esp = p4.enter_context(tc.tile_pool(name="esp", bufs=1))
            accp = p4.enter_context(tc.tile_pool(name="accp", bufs=1))
            g_ps = p4.enter_context(tc.tile_pool(name="g_ps", bufs=1, space="PSUM"))

            acc = {
                d: accp.tile([128, MB, PC], f32, name=f"acc{d}")
                for d in range(ND)
            }

            def los_of(d):
                return G0 if d in (0, ND - 1) else [0] * MB

            dv_ps = g_ps.tile([128, 512], f32, name="dv_ps", tag="gp7")
            uc = 0
            csts = {}
            rs0 = None
            csjs = {}

            def scales_block():
                # rowscale: rs = invdv(my col block) * inv_de
                nonlocal rs0
                d1 = gw.tile([128, MB], f32, name="d1")
                nc.vector.tensor_scalar_max(d1[:], dv_sb[:], 1.0)
                sq = gw.tile([128, MB], f32, name="sqv")
                nc.scalar.sqrt(sq[:], d1[:])
                rc = gw.tile([128, MB], f32, name="rc")
                nc.vector.reciprocal(rc[:], sq[:])
                mk = gw.tile([128, MB], f32, name="mk")
                nc.vector.tensor_scalar(mk[:], dv_sb[:], 0.0, None, op0=Alu.is_gt)
                iv = gw.tile([128, MB], f32, name="iv")
                nc.vector.tensor_tensor(iv[:], rc[:], mk[:], op=Alu.mult)
                rs0 = gw.tile([128, MB], f32, name="rsu0")
                nc.vector.tensor_scalar_mul(rs0[:], iv[:], inv_de)

                # colscale source: cs = invdv over all N (from dv AllGather)
                q = N // 128
                dvw = gw.tile([128, q], f32, name="dvw")
                nc.scalar.dma_start(
                    dvw[:], dv_full.rearrange("(cm p) -> p cm", p=128)
                )
                d1w = gw.tile([128, q], f32, name="d1w")
                nc.vector.tensor_scalar_max(d1w[:], dvw[:], 1.0)
                sqw = gw.tile([128, q], f32, name="sqw")
                nc.scalar.sqrt(sqw[:], d1w[:])
                rcw = gw.tile([128, q], f32, name="rcw")
                nc.vector.reciprocal(rcw[:], sqw[:])
                mkw = gw.tile([128, q], f32, name="mkw")
                nc.vector.tensor_scalar(mkw[:], dvw[:], 0.0, None, op0=Alu.is_gt)
                ivw = gw.tile([128, q], f32, name="ivw")
                nc.vector.tensor_tensor(ivw[:], rcw[:], mkw[:], op=Alu.mult)
                nc.scalar.dma_start(
                    cs_dram.rearrange("(cm p) -> p cm", p=128), ivw[:]
                )
                cs2d = cs_dram.rearrange("(a n) -> a n", a=1)
                for dd in range(ND):
                    jra = nc.scalar.alloc_register(f"jra{dd}")
                    nc.scalar.reg_load(jra, jsel_sb[0:1, dd : dd + 1])
                    jda = nc.scalar.snap(
                        jra, donate=True, min_val=0, max_val=NCORES - 1
                    )
                    cs_t = gw.tile([1, RP], f32, name="cs_t", tag=f"cs_t{dd % 2}")
                    nc.scalar.dma_start(cs_t[:], cs2d[:, bass.ds(jda * RP, RP)])
                    csts[dd] = cs_t

            def output_stage(d):
                nonlocal uc
                los = los_of(d)
                csj = csp.tile([128, PAD], f32, name="csj", tag="csj")
                for hh in range(2):
                    cps = g_ps.tile([128, 512], f32, name="cps", tag=f"gp{uc % 7}")
                    uc += 1
                    nc.tensor.matmul(
                        cps[:], ones1[:],
                        csts[d][:, hh * 512 : (hh + 1) * 512],
                        start=True, stop=True,
                    )
                    nc.scalar.copy(csj[:, hh * 512 : (hh + 1) * 512], cps[:])
                nc.vector.memset(csj[:, RP:PAD], 0.0)
                csj3 = csj.rearrange("p (t u) -> p t u", u=3)
                for m in range(MB):
                    lo = los[m]
                    es = acc[d][:, m, :]
                    z2 = esp.tile([128, PC], f32, name="z2", tag="z2")
                    nc.vector.tensor_scalar(
                        z2[:, lo:], es[:, lo:], 1.0 / B2, CB2,
                        op0=Alu.mult, op1=Alu.add,
                    )
                    m2 = esp.tile([128, PC], f32, name="m2", tag="m2")
                    nc.vector.tensor_scalar(
                        m2[:, lo:], z2[:, lo:], MAGIC, MAGIC,
                        op0=Alu.add, op1=Alu.subtract,
                    )
                    r2 = esp.tile([128, PC], f32, name="r2", tag="r2")
                    nc.vector.scalar_tensor_tensor(
                        r2[:, lo:], in0=m2[:, lo:], scalar=-B2, in1=es[:, lo:],
                        op0=Alu.mult, op1=Alu.add,
                    )
                    z1 = esp.tile([128, PC], f32, name="z1", tag="z1")
                    nc.vector.tensor_scalar(
                        z1[:, lo:], r2[:, lo:], 1.0 / BASE, CB1,
                        op0=Alu.mult, op1=Alu.add,
                    )
                    m1 = esp.tile([128, PC], f32, name="m1", tag="m1")
                    nc.vector.tensor_scalar(
                        m1[:, lo:], z1[:, lo:], MAGIC, MAGIC,
                        op0=Alu.add, op1=Alu.subtract,
                    )
                    m0 = esp.tile([128, PC], f32, name="m0", tag="m0")
                    nc.vector.scalar_tensor_tensor(
                        m0[:, lo:], in0=m1[:, lo:], scalar=-BASE, in1=r2[:, lo:],
                        op0=Alu.mult, op1=Alu.add,
                    )
                    gs = stg.tile([128, PAD], f32, name="gs", tag="gs")
                    gs3 = gs.rearrange("p (t u) -> p t u", u=3)
                    for u, mu in ((2, m2), (1, m1), (0, m0)):
                        au = esp.tile([128, PC], f32, name=f"a{u}", tag="au")
                        nc.scalar.activation(
                            au[:, lo:], mu[:, lo:], Act.Copy,
                            scale=rs0[:, m : m + 1],
                        )
                        nc.vector.tensor_tensor(
                            gs3[:, lo:, u], au[:, lo:], csj3[:, lo:, u],
                            op=Alu.mult,
                        )
                    nc.sync.dma_start(
                        g_out[
                            m * 128 : (m + 1) * 128,
                            d * RP + 3 * lo : (d + 1) * RP,
                        ],
                        gs[:, 3 * lo : RP],
                    )

            for pp in range(NP):
                ltr = pre0["ltr"] if pp == 0 else load_ltr(pp)
                # dv first: pair-3's dv only needs lt3, so the dv AllGather
                # can run ahead of the pair-3 pk AllGather in the CC stream
                for o in range(NCORES):
                    for m in range(MB):
                        nc.tensor.matmul(
                            dv_ps[:, m : m + 1],
                            ltr[:, :, o, m * 128 : (m + 1) * 128],
                            ones_dr[:, :, 0:1],
                            perf_mode=mybir.MatmulPerfMode.DoubleRow,
                            start=(pp == 0 and o == 0 and m == 0),
                            stop=(
                                pp == NP - 1 and o == NCORES - 1 and m == MB - 1
                            ),
                            skip_group_check=True,
                        )
                if pp == NP - 1:
                    dv_sb = gw.tile([128, MB], f32, name="dv_sb")
                    nc.vector.tensor_copy(dv_sb[:], dv_ps[:, 0:MB])
                    nc.scalar.dma_start(
                        dv_my_d.rearrange("(m p) -> p m", p=128), dv_sb[:]
                    )
                    nc.gpsimd.collective_compute(
                        "AllGather", Alu.bypass, replica_groups=rg,
                        ins=[dv_my_d.opt()], outs=[dv_full.opt()],
                    )
                    cc_pk5_fn(NP - 1)
                    scales_block()
                if pp == 0:
                    rhs = {0: pre0[0], 1: pre0[1]}
                else:
                    rhs = {0: load_rh5(pp, 0), 1: load_rh5(pp, 1)}
                for d in range(ND):
                    if d + 2 < ND:
                        rhs[d + 2] = load_rh5(pp, d + 2)
                    los = los_of(d)
                    rh = rhs.pop(d)
                    for m in range(MB):
                        lo = los[m]
                        ps = g_ps.tile(
                            [128, 512], f32, name="gps", tag=f"gp{uc % 7}"
                        )
                        uc += 1
                        k = 0
                        for i in range(2):
                            for o in range(NCORES):
                                nc.tensor.matmul(
                                    ps[:, 0 : PC - lo],
                                    ltr[:, i, o, m * 128 : (m + 1) * 128],
                                    rh[:, i, o, lo:PC],
                                    start=(k == 0),
                                    stop=(k == 15),
                                    skip_group_check=True,
                                )
                                k += 1
                        asl = acc[d][:, m, lo:PC]
                        if pp == 0:
                            nc.scalar.copy(asl, ps[:, 0 : PC - lo])
                        else:
                            nc.vector.tensor_tensor(
                                asl, asl, ps[:, 0 : PC - lo], op=Alu.add
                            )
                    if pp == NP - 1:
                        # acc[d] is final: emit its output stage now so the
                        # unpack/scale work overlaps the remaining units
                        output_stage(d)

    nc.compile()
    return nc


_CACHE = {}


def get_nc(N, D, KN, NCORES):
    key = (N, D, KN, NCORES)
    if key not in _CACHE:
        _CACHE[key] = build_nc(N, D, KN, NCORES)
    return _CACHE[key]


def kernel(feats, kn, _trace=False):
    feats = np.asarray(feats, dtype=np.float32)
    kn = int(kn)
    N, D = feats.shape
    NCORES = 8
    ND = 5
    RP = N // NCORES
    nc = get_nc(N, D, kn, NCORES)
    ident = np.eye(128, dtype=np.float32)
    in_maps = []
    for c in range(NCORES):
        jsel = np.zeros((1, 8), np.int32)
        for d in range(8):
            jsel[0, d] = (c + d) % NCORES
        in_maps.append({
            "feats_all": feats,
            "feats_my": feats[c * RP : (c + 1) * RP],
            "ident_in": ident,
            "jsel_in": jsel,
        })
    res = run_bass_kernel_spmd(
        nc, in_maps, core_ids=list(range(NCORES)), trace=_trace
    )
    out = np.empty((N, N), dtype=np.float32)
    res_g = [res.results[c]["g_out"] for c in range(NCORES)]  # [RP, ND*RP]
    for c in range(NCORES):
        g = res_g[c]
        for d in range(ND):
            j = (c + d) % NCORES
            blk = g[:, d * RP : (d + 1) * RP]
            if d == 0:
                # kernel computed upper triangle only; mirror below diag
                full = np.triu(blk) + np.triu(blk, 1).T
                out[c * RP : (c + 1) * RP, c * RP : (c + 1) * RP] = full
            elif d == ND - 1:
                if c >= NCORES // 2:
                    continue
                # core c has triu of (c,j); core j has triu of (j,c) = tril^T
                blk2 = res_g[j][:, (ND - 1) * RP : ND * RP]
                full = np.triu(blk) + np.tril(blk2.T, -1)
                out[c * RP : (c + 1) * RP, j * RP : (j + 1) * RP] = full
                out[j * RP : (j + 1) * RP, c * RP : (c + 1) * RP] = full.T
            else:
                out[c * RP : (c + 1) * RP, j * RP : (j + 1) * RP] = blk
                out[j * RP : (j + 1) * RP, c * RP : (c + 1) * RP] = blk.T
    if _trace:
        return out, res
    return out


if __name__ == "__main__":
    inputs = {
        "feats": np.load("/tmp/feats.npy"),
        "kn": 10,
    }
    out = kernel(**inputs)
    print("out", out.shape, out.dtype, float(np.abs(out).max()))



# revision 54
# speedup vs baseline: 1.5245x; 1.0846x over previous
"""Trainium2 Bass kernel for nn_Attention_16011638079620 (gnn_message_passing).

Computes, for feats [8192, 256] f32 and kn=10:
    sim   = cosine-similarity(feats)            [N, N]
    B     = rowwise top-kn one-hot mask of softmax(sim) (rank-preserving)
    G     = (1/kn) * invdv_i * invdv_j * (B^T B)_ij,  dv = colsums of B

Strategy (8 cores):
  - sim via 3-pass bf16 hi/lo split matmuls (exact top-k, 4x faster than f32)
  - B columns packed 3-per-fp16 value (base 24; counts <= 23 so the packed
    matmul B^T @ packedB is integer-exact in fp32 PSUM) -> 1.5x fp8-DR rate
  - G is symmetric: core c computes blocks (c, (c+d)%8) for d=0..4; host
    mirrors the rest. lhsT is always the core's own column slice (AllToAll).
  - dv via fp8-DoubleRow ones-matmuls, interleaved with the sim phase.
  - software pipeline: mask/pack of block m-1 runs behind sim of block m so
    PSUM banks release early; collectives merged in block pairs.
"""

import sys

sys.path.insert(0, "/opt/trn_rl_repo")

from contextlib import ExitStack

import numpy as np

import concourse.bass as bass
import concourse.tile as tile
from concourse import bacc, mybir
from concourse.bass import _add_dep_helper
from concourse.bass_utils import run_bass_kernel_spmd

f32 = mybir.dt.float32
bf16 = mybir.dt.bfloat16
fp16 = mybir.dt.float16
fp8 = mybir.dt.float8e4
i32 = mybir.dt.int32
Alu = mybir.AluOpType
Act = mybir.ActivationFunctionType
NEG = -1e30
BASE = 24.0
B2 = BASE * BASE  # 576
MAGIC = 12582912.0  # 1.5 * 2**23: (z + MAGIC) - MAGIC == round-to-nearest(z)
CB2 = 0.5 / B2 - 0.5  # bias so round(P/576 + CB2) == floor(P/576) exactly
CB1 = 0.5 / BASE - 0.5


def build_nc(N, D, KN, NCORES):
    RP = N // NCORES           # 1024 rows/G-rows per core
    MB = RP // 128             # 8 row blocks per core
    NCH = N // 512             # topk chunks
    DT = D // 128              # 2 feature chunks
    PC = 342                   # packed cols per 1024-col slice (342*3 = 1026)
    PCW = NCORES * PC          # 2736 packed cols total per row
    PAD = 3 * PC               # 1026 padded cols per slice
    BW = NCORES * PAD          # 8208 padded mask width
    KK = N // 128              # 64 contraction chunks for phase D
    NP = MB // 2               # 4 block pairs for collectives
    ND = 5                     # symmetric blocks per core
    assert 8 < KN <= 16

    inv_de = float(np.float32(1.0) / np.float32(KN))

    nc = bacc.Bacc(
        "TRN2",
        target_bir_lowering=False,
        debug=False,
        enable_asserts=False,
        num_devices=NCORES,
    )
    feats_all = nc.dram_tensor("feats_all", [N, D], f32, kind="ExternalInput").ap()
    feats_my = nc.dram_tensor("feats_my", [RP, D], f32, kind="ExternalInput").ap()
    ident_in = nc.dram_tensor("ident_in", [128, 128], f32, kind="ExternalInput").ap()
    jsel_in = nc.dram_tensor("jsel_in", [1, 8], i32, kind="ExternalInput").ap()
    g_out = nc.dram_tensor("g_out", [RP, ND * RP], f32, kind="ExternalOutput").ap()

    rg = [list(range(NCORES))]

    with tile.TileContext(nc) as tc, ExitStack() as ctx:
        dram = ctx.enter_context(tc.tile_pool(name="dram", bufs=1, space="DRAM"))
        b_grp2 = [
            dram.tile([NCORES, 2, 128, RP], fp8, name=f"b_grp2_{p}") for p in range(NP)
        ]
        lts_d2 = [
            dram.tile([NCORES, 2, 128, RP], fp8, name=f"lts_d2_{p}") for p in range(NP)
        ]
        # pk exchange is an AllGather: one cheap contiguous write per block
        # (A2A variants need 5x duplicated payload staging, which drowns the
        # sync engine in DIRECT2D descriptor work and stalls the sim phase)
        pk_in2 = [dram.tile([2, 128, PCW], fp16, name=f"pk_in2_{p}") for p in range(NP)]
        pk_ag2 = [
            dram.tile(
                [NCORES, 2, 128, PCW], fp16, addr_space="Shared", name=f"pk_ag2_{p}"
            )
            for p in range(NP)
        ]
        dv_my_d = dram.tile([RP], f32, name="dv_my_d")
        dv_full = dram.tile([N], f32, addr_space="Shared", name="dv_full")
        cs_dram = dram.tile([N], f32, name="cs_dram")

        pers = ctx.enter_context(tc.tile_pool(name="pers", bufs=1))
        dv_ps = None  # allocated from g_ps during the G d0 window

        ident = pers.tile([128, 128], f32, name="ident")
        nc.sync.dma_start(ident[:], ident_in)
        idb = pers.tile([128, 128], bf16, name="idb")
        nc.vector.tensor_copy(idb[:], ident[:])
        jsel_sb = pers.tile([1, 8], i32, name="jsel_sb")
        nc.sync.dma_start(jsel_sb[:], jsel_in)
        ones1 = pers.tile([1, 128], f32, name="ones1")
        nc.vector.memset(ones1[:], 1.0)
        zlh = pers.tile([1, 128], fp16, name="zlh")
        nc.vector.memset(zlh[:], 0.0)
        zrh = pers.tile([1, 512], fp16, name="zrh")
        nc.vector.memset(zrh[:], 0.0)
        ones_dr = pers.tile([128, 2, 16], fp8, name="ones_dr")
        nc.vector.memset(ones_dr[:], 1.0)

        # lhsT (own dense column slice) is loaded per pair in phase D.
        # These pools live at ctx scope so pair-0's loads can be issued
        # from inside the sim loop (ahead of mask_pack(7)'s DMA writes on
        # the in-order sync queue).
        rhp = ctx.enter_context(tc.tile_pool(name="rhp", bufs=3))
        ltrp = ctx.enter_context(tc.tile_pool(name="ltrp", bufs=2))
        jds = {}
        for dd in range(ND):
            jr = nc.sync.alloc_register(f"jr{dd}")
            nc.sync.reg_load(jr, jsel_sb[0:1, dd : dd + 1])
            jds[dd] = nc.sync.snap(jr, donate=True, min_val=0, max_val=NCORES - 1)

        def load_rh5(pp, d):
            rh = rhp.tile([128, 2, NCORES, PC], fp16, name="rh5", tag="rh5")
            for i in range(2):
                nc.sync.dma_start(
                    rh[:, i],
                    pk_ag2[pp][:, i, :, :].rearrange("o p c -> p o c")[
                        :, :, bass.ds(jds[d] * PC, PC)
                    ],
                )
            return rh

        def load_ltr(pp):
            ltr = ltrp.tile([128, 2, NCORES, RP], fp8, name="ltr", tag="ltr")
            for ii in range(2):
                nc.sync.dma_start(
                    ltr[:, ii],
                    lts_d2[pp][:, ii].rearrange("o p q -> p o q"),
                )
            return ltr

        pre0 = {}

        # ---------------- phase 1: normalize + hi/lo split + transpose ------
        with ExitStack() as p12:
            fsb = p12.enter_context(tc.tile_pool(name="fsb", bufs=1))
            fnt_hi = [fsb.tile([128, N], bf16, name=f"fh{h}") for h in range(DT)]
            fnt_lo = [fsb.tile([128, N], bf16, name=f"fl{h}") for h in range(DT)]
            fnt_myh = [fsb.tile([128, RP], bf16, name=f"fmh{h}") for h in range(DT)]
            fnt_myl = [fsb.tile([128, RP], bf16, name=f"fml{h}") for h in range(DT)]

            with ExitStack() as p1:
                wrk = p1.enter_context(tc.tile_pool(name="wrk", bufs=3))
                sml = p1.enter_context(tc.tile_pool(name="sml", bufs=6))
                tp_ps = p1.enter_context(
                    tc.tile_pool(name="tp_ps", bufs=2, space="PSUM")
                )

                def norm_group(src4, dh, dl, col0, nb):
                    # nb row-blocks batched: one op set for the whole group
                    ft4 = wrk.tile([128, nb, D], f32, name="ft4")
                    nc.sync.dma_start(ft4[:], src4)
                    tps = {}
                    for x in range(2):
                        tps[x] = tp_ps.tile(
                            [128, DT, nb * 128], bf16, name=f"tp{x}", tag=f"tp{x}"
                        )
                    sq4 = wrk.tile([128, nb, D], f32, name="sq4")
                    nc.scalar.square(
                        sq4.rearrange("p b d -> p (b d)"),
                        ft4.rearrange("p b d -> p (b d)"),
                    )
                    n24 = sml.tile([128, nb, 1], f32, name="n24")
                    nc.vector.reduce_sum(n24[:], sq4[:], axis=mybir.AxisListType.X)
                    nrm4 = sml.tile([128, nb, 1], f32, name="nrm4")
                    nc.scalar.sqrt(
                        nrm4.rearrange("p b o -> p (b o)"),
                        n24.rearrange("p b o -> p (b o)"),
                    )
                    inv4 = sml.tile([128, nb, 1], f32, name="inv4")
                    nc.vector.reciprocal(
                        inv4.rearrange("p b o -> p (b o)"),
                        nrm4.rearrange("p b o -> p (b o)"),
                    )
                    fn4 = wrk.tile([128, nb, D], f32, name="fn4")
                    nc.vector.tensor_tensor(
                        fn4[:], ft4[:], inv4[:].broadcast_to([128, nb, D]),
                        op=Alu.mult,
                    )
                    fh4 = wrk.tile([128, nb, D], bf16, name="fh4")
                    nc.scalar.copy(
                        fh4.rearrange("p b d -> p (b d)"),
                        fn4.rearrange("p b d -> p (b d)"),
                    )
                    fl4 = wrk.tile([128, nb, D], bf16, name="fl4")
                    nc.vector.tensor_tensor(
                        fl4[:], fn4[:], fh4[:], op=Alu.subtract
                    )
                    for i in range(nb):
                        for h in range(DT):
                            for x, s4 in ((0, fh4), (1, fl4)):
                                nc.tensor.transpose(
                                    tps[x][:, h, i * 128 : (i + 1) * 128],
                                    s4[:, i, h * 128 : (h + 1) * 128],
                                    idb[:],
                                )
                    for h in range(DT):
                        for x, dst in ((0, dh), (1, dl)):
                            nc.scalar.copy(
                                dst[h][:, col0 : col0 + nb * 128], tps[x][:, h, :]
                            )

                fm4 = feats_my.rearrange("(g i p) d -> g p i d", p=128, i=4)
                for g in range(MB // 4):
                    norm_group(fm4[g], fnt_myh, fnt_myl, g * 512, 4)
                fa4 = feats_all.rearrange("(g i p) d -> g p i d", p=128, i=4)
                for g in range(N // 512):
                    norm_group(fa4[g], fnt_hi, fnt_lo, g * 512, 4)

            # ---------------- phase 2: sim, topk, mask, pack, CC -----------
            with ExitStack() as p2:
                simp = p2.enter_context(tc.tile_pool(name="simp", bufs=3))
                smal = p2.enter_context(tc.tile_pool(name="smal", bufs=2))
                bmpp = p2.enter_context(tc.tile_pool(name="bmpp", bufs=1))
                pkp = p2.enter_context(tc.tile_pool(name="pkp", bufs=1))
                t0p = p2.enter_context(tc.tile_pool(name="t0p", bufs=2))
                sim_ps = p2.enter_context(
                    tc.tile_pool(name="sim_ps", bufs=1, space="PSUM")
                )
                combos = []
                for h in range(DT):
                    combos.append((fnt_myh[h], fnt_hi[h]))
                    combos.append((fnt_myh[h], fnt_lo[h]))
                    combos.append((fnt_myl[h], fnt_hi[h]))

                tkns = {}
                halves = {}

                def sim_block(m):
                    sh0 = simp.tile([128, N // 2], f32, name="sh0", tag="sh")
                    sh1 = simp.tile([128, N // 2], f32, name="sh1", tag="sh")
                    halves[m] = (sh0, sh1)
                    cand = smal.tile([128, 8 * NCH], f32, name="cand", tag="cand")
                    for qr in range(4):
                        pss = [
                            sim_ps.tile([128, 512], f32, name=f"sq{t}", tag=f"sq{t}")
                            for t in range(4)
                        ]
                        for ci, (la, ra) in enumerate(combos):
                            lt = la[:, m * 128 : (m + 1) * 128]
                            for t in range(4):
                                ntc = qr * 4 + t
                                nc.tensor.matmul(
                                    pss[t][:],
                                    lt,
                                    ra[:, ntc * 512 : (ntc + 1) * 512],
                                    start=(ci == 0),
                                    stop=(ci == 5),
                                )
                        sh = (sh0, sh1)[qr // 2]
                        for t in range(4):
                            ntc = qr * 4 + t
                            nc.vector.max(
                                cand[:, ntc * 8 : (ntc + 1) * 8], pss[t][:]
                            )
                            nc.scalar.copy(
                                sh[:, (ntc % 8) * 512 : (ntc % 8 + 1) * 512],
                                pss[t][:],
                            )
                    c8 = smal.tile([128, 8], f32, name="c8", tag="c8")
                    nc.vector.max(c8[:], cand[:])
                    cand2 = smal.tile([128, 8 * NCH], f32, name="cand2", tag="cand2")
                    nc.vector.match_replace(cand2[:], c8[:], cand[:], NEG)
                    c8b = smal.tile([128, 8], f32, name="c8b", tag="c8b")
                    nc.vector.max(c8b[:], cand2[:])
                    tkns[m] = c8b

                def mask_pack(m):
                    tkn = tkns[m][:, KN - 9 : KN - 8]
                    sh0, sh1 = halves[m]
                    bmp = bmpp.tile([128, BW], fp8, name="bmp")
                    for j in range(NCORES):
                        sh = (sh0, sh1)[j // 4]
                        nc.vector.tensor_scalar(
                            bmp[:, j * PAD : j * PAD + RP],
                            sh[:, (j % 4) * RP : (j % 4 + 1) * RP],
                            tkn,
                            None,
                            op0=Alu.is_ge,
                        )
                        nc.vector.memset(bmp[:, j * PAD + RP : (j + 1) * PAD], 0.0)
                    pk = pkp.tile([128, PCW], fp16, name="pk")
                    bm3 = bmp.rearrange("p (j t u) -> p j t u", j=NCORES, u=3)
                    for j in range(NCORES):
                        t0 = t0p.tile([128, PC], f32, name="t0")
                        nc.vector.scalar_tensor_tensor(
                            t0[:], in0=bm3[:, j, :, 1], scalar=BASE,
                            in1=bm3[:, j, :, 0], op0=Alu.mult, op1=Alu.add,
                        )
                        nc.vector.scalar_tensor_tensor(
                            pk[:, j * PC : (j + 1) * PC],
                            in0=bm3[:, j, :, 2], scalar=B2, in1=t0[:],
                            op0=Alu.mult, op1=Alu.add,
                        )
                    pp, i = m // 2, m % 2
                    nc.sync.dma_start(pk_in2[pp][i], pk[:])
                    nc.sync.dma_start(
                        b_grp2[pp][:, i].rearrange("j p q -> p j q"),
                        bmp.rearrange("p (j q) -> p j q", j=NCORES)[:, :, 0:RP],
                    )

                def cc_pk5(pp):
                    nc.gpsimd.collective_compute(
                        "AllGather", Alu.bypass, replica_groups=rg,
                        ins=[pk_in2[pp].opt()], outs=[pk_ag2[pp].opt()],
                    )

                def cc_pair(pp, defer_pk=False):
                    # small lt A2A first so dv/G lhs unblocks earliest
                    nc.gpsimd.collective_compute(
                        "AllToAll", Alu.bypass, replica_groups=rg,
                        ins=[b_grp2[pp].opt()], outs=[lts_d2[pp].opt()],
                    )
                    if not defer_pk:
                        cc_pk5(pp)

                for m in range(MB):
                    sim_block(m)
                    if m >= 1:
                        mask_pack(m - 1)
                        if (m - 1) % 2 == 1:
                            cc_pair((m - 1) // 2)
                # prefetch pair-0's G operands ahead of mask_pack(7)'s
                # writes (pair-3's AG is deferred, so the small delay to
                # pk_in2[3]/b_grp2[3] is harmless)
                pre0["ltr"] = load_ltr(0)
                pre0[0] = load_rh5(0, 0)
                pre0[1] = load_rh5(0, 1)
                mask_pack(MB - 1)
                cc_pair(NP - 1)

        # ---------------- phase 3+4: pair-major G, SBUF accumulation --------
        # G units (pair, d, m) run as each pair's collectives land, so PE
        # work rides inside the CC windows; per-(d,m) psum partials are
        # added into SBUF accumulators and only pair-3's sweep + the output
        # stage remain after the last arrival.
        G0 = [(m * 128) // 3 for m in range(MB)]
        with ExitStack() as p4:
            gw = p4.enter_context(tc.tile_pool(name="gw", bufs=4))
            csp = p4.enter_context(tc.tile_pool(name="csp", bufs=2))
            stg = p4.enter_context(tc.tile_pool(name="stg", bufs=2))
            esp = p4.enter_context(tc.tile_pool(name="esp", bufs=1))
            accp = p4.enter_context(tc.tile_pool(name="accp", bufs=1))
            g_ps = p4.enter_context(tc.tile_pool(name="g_ps", bufs=1, space="PSUM"))

            acc = {
                d: accp.tile([128, MB, PC], f32, name=f"acc{d}")
                for d in range(ND)
            }

            def los_of(d):
                return G0 if d in (0, ND - 1) else [0] * MB

            dv_ps = g_ps.tile([128, 512], f32, name="dv_ps", tag="gp7")
            uc = 0
            csts = {}
            rs0 = None
            csjs = {}

            def rowscale_block():
                # rowscale: rs = invdv(my col block) * inv_de (dv_sb only)
                nonlocal rs0
                d1 = gw.tile([128, MB], f32, name="d1")
                nc.vector.tensor_scalar_max(d1[:], dv_sb[:], 1.0)
                sq = gw.tile([128, MB], f32, name="sqv")
                nc.scalar.sqrt(sq[:], d1[:])
                rc = gw.tile([128, MB], f32, name="rc")
                nc.vector.reciprocal(rc[:], sq[:])
                mk = gw.tile([128, MB], f32, name="mk")
                nc.vector.tensor_scalar(mk[:], dv_sb[:], 0.0, None, op0=Alu.is_gt)
                iv = gw.tile([128, MB], f32, name="iv")
                nc.vector.tensor_tensor(iv[:], rc[:], mk[:], op=Alu.mult)
                rs0 = gw.tile([128, MB], f32, name="rsu0")
                nc.vector.tensor_scalar_mul(rs0[:], iv[:], inv_de)

            def colscale_block():
                # colscale source: cs = invdv over all N (needs dv AllGather)
                q = N // 128
                dvw = gw.tile([128, q], f32, name="dvw")
                nc.scalar.dma_start(
                    dvw[:], dv_full.rearrange("(cm p) -> p cm", p=128)
                )
                d1w = gw.tile([128, q], f32, name="d1w")
                nc.vector.tensor_scalar_max(d1w[:], dvw[:], 1.0)
                sqw = gw.tile([128, q], f32, name="sqw")
                nc.scalar.sqrt(sqw[:], d1w[:])
                rcw = gw.tile([128, q], f32, name="rcw")
                nc.vector.reciprocal(rcw[:], sqw[:])
                mkw = gw.tile([128, q], f32, name="mkw")
                nc.vector.tensor_scalar(mkw[:], dvw[:], 0.0, None, op0=Alu.is_gt)
                ivw = gw.tile([128, q], f32, name="ivw")
                nc.vector.tensor_tensor(ivw[:], rcw[:], mkw[:], op=Alu.mult)
                nc.scalar.dma_start(
                    cs_dram.rearrange("(cm p) -> p cm", p=128), ivw[:]
                )
                cs2d = cs_dram.rearrange("(a n) -> a n", a=1)
                for dd in range(ND):
                    jra = nc.scalar.alloc_register(f"jra{dd}")
                    nc.scalar.reg_load(jra, jsel_sb[0:1, dd : dd + 1])
                    jda = nc.scalar.snap(
                        jra, donate=True, min_val=0, max_val=NCORES - 1
                    )
                    cs_t = gw.tile([1, RP], f32, name="cs_t", tag=f"cs_t{dd % 2}")
                    nc.scalar.dma_start(cs_t[:], cs2d[:, bass.ds(jda * RP, RP)])
                    csts[dd] = cs_t

            def output_stage(d):
                nonlocal uc
                los = los_of(d)
                csj = csp.tile([128, PAD], f32, name="csj", tag="csj")
                for hh in range(2):
                    cps = g_ps.tile([128, 512], f32, name="cps", tag=f"gp{uc % 7}")
                    uc += 1
                    nc.tensor.matmul(
                        cps[:], ones1[:],
                        csts[d][:, hh * 512 : (hh + 1) * 512],
                        start=True, stop=True,
                    )
                    nc.scalar.copy(csj[:, hh * 512 : (hh + 1) * 512], cps[:])
                nc.vector.memset(csj[:, RP:PAD], 0.0)
                csj3 = csj.rearrange("p (t u) -> p t u", u=3)
                for m in range(MB):
                    lo = los[m]
                    es = acc[d][:, m, :]
                    z2 = esp.tile([128, PC], f32, name="z2", tag="z2")
                    nc.vector.tensor_scalar(
                        z2[:, lo:], es[:, lo:], 1.0 / B2, CB2,
                        op0=Alu.mult, op1=Alu.add,
                    )
                    m2 = esp.tile([128, PC], f32, name="m2", tag="m2")
                    nc.vector.tensor_scalar(
                        m2[:, lo:], z2[:, lo:], MAGIC, MAGIC,
                        op0=Alu.add, op1=Alu.subtract,
                    )
                    r2 = esp.tile([128, PC], f32, name="r2", tag="r2")
                    nc.vector.scalar_tensor_tensor(
                        r2[:, lo:], in0=m2[:, lo:], scalar=-B2, in1=es[:, lo:],
                        op0=Alu.mult, op1=Alu.add,
                    )
                    z1 = esp.tile([128, PC], f32, name="z1", tag="z1")
                    nc.vector.tensor_scalar(
                        z1[:, lo:], r2[:, lo:], 1.0 / BASE, CB1,
                        op0=Alu.mult, op1=Alu.add,
                    )
                    m1 = esp.tile([128, PC], f32, name="m1", tag="m1")
                    nc.vector.tensor_scalar(
                        m1[:, lo:], z1[:, lo:], MAGIC, MAGIC,
                        op0=Alu.add, op1=Alu.subtract,
                    )
                    m0 = esp.tile([128, PC], f32, name="m0", tag="m0")
                    nc.vector.scalar_tensor_tensor(
                        m0[:, lo:], in0=m1[:, lo:], scalar=-BASE, in1=r2[:, lo:],
                        op0=Alu.mult, op1=Alu.add,
                    )
                    gs = stg.tile([128, PAD], f32, name="gs", tag="gs")
                    gs3 = gs.rearrange("p (t u) -> p t u", u=3)
                    for u, mu in ((2, m2), (1, m1), (0, m0)):
                        au = esp.tile([128, PC], f32, name=f"a{u}", tag="au")
                        nc.scalar.activation(
                            au[:, lo:], mu[:, lo:], Act.Copy,
                            scale=rs0[:, m : m + 1],
                        )
                        nc.vector.tensor_tensor(
                            gs3[:, lo:, u], au[:, lo:], csj3[:, lo:, u],
                            op=Alu.mult,
                        )
                    nc.sync.dma_start(
                        g_out[
                            m * 128 : (m + 1) * 128,
                            d * RP + 3 * lo : (d + 1) * RP,
                        ],
                        gs[:, 3 * lo : RP],
                    )

            ltrs = {}

            def dv_block(pp):
                for o in range(NCORES):
                    for m in range(MB):
                        nc.tensor.matmul(
                            dv_ps[:, m : m + 1],
                            ltrs[pp][:, :, o, m * 128 : (m + 1) * 128],
                            ones_dr[:, :, 0:1],
                            perf_mode=mybir.MatmulPerfMode.DoubleRow,
                            start=(pp == 0 and o == 0 and m == 0),
                            stop=(
                                pp == NP - 1 and o == NCORES - 1 and m == MB - 1
                            ),
                            skip_group_check=True,
                        )

            for pp in range(NP):
                if pp not in ltrs:
                    ltrs[pp] = pre0["ltr"] if pp == 0 else load_ltr(pp)
                ltr = ltrs[pp]
                if pp < 2:
                    dv_block(pp)
                elif pp == 2:
                    # finish ALL dv here: dv3 only needs lt3 (lands with
                    # AG2), so the dv AllGather can trail AG3 closely
                    # instead of waiting for pairs 0-2's PE sweeps
                    dv_block(2)
                    ltrs[3] = load_ltr(3)
                    dv_block(3)
                    dv_sb = gw.tile([128, MB], f32, name="dv_sb")
                    nc.vector.tensor_copy(dv_sb[:], dv_ps[:, 0:MB])
                    nc.scalar.dma_start(
                        dv_my_d.rearrange("(m p) -> p m", p=128), dv_sb[:]
                    )
                    nc.gpsimd.collective_compute(
                        "AllGather", Alu.bypass, replica_groups=rg,
                        ins=[dv_my_d.opt()], outs=[dv_full.opt()],
                    )
                    rowscale_block()
                else:
                    colscale_block()
                if pp == 0:
                    rhs = {0: pre0[0], 1: pre0[1]}
                else:
                    rhs = {0: load_rh5(pp, 0), 1: load_rh5(pp, 1)}
                for d in range(ND):
                    if d + 2 < ND:
                        rhs[d + 2] = load_rh5(pp, d + 2)
                    los = los_of(d)
                    rh = rhs.pop(d)
                    for m in range(MB):
                        lo = los[m]
                        ps = g_ps.tile(
                            [128, 512], f32, name="gps", tag=f"gp{uc % 7}"
                        )
                        uc += 1
                        k = 0
                        for i in range(2):
                            for o in range(NCORES):
                                nc.tensor.matmul(
                                    ps[:, 0 : PC - lo],
                                    ltr[:, i, o, m * 128 : (m + 1) * 128],
                                    rh[:, i, o, lo:PC],
                                    start=(k == 0),
                                    stop=(k == 15),
                                    skip_group_check=True,
                                )
                                k += 1
                        asl = acc[d][:, m, lo:PC]
                        if pp == 0:
                            nc.scalar.copy(asl, ps[:, 0 : PC - lo])
                        else:
                            nc.vector.tensor_tensor(
                                asl, asl, ps[:, 0 : PC - lo], op=Alu.add
                            )
                    if pp == NP - 1:
                        # acc[d] is final: emit its output stage now so the
                        # unpack/scale work overlaps the remaining units
                        output_stage(d)

    nc.compile()
    return nc


_CACHE = {}


def get_nc(N, D, KN, NCORES):
    key = (N, D, KN, NCORES)
    if key not in _CACHE:
        _CACHE[key] = build_nc(N, D, KN, NCORES)
    return _CACHE[key]


def kernel(feats, kn, _trace=False):
    feats = np.asarray(feats, dtype=np.float32)
    kn = int(kn)
    N, D = feats.shape
    NCORES = 8
    ND = 5
    RP = N // NCORES
    nc = get_nc(N, D, kn, NCORES)
    ident = np.eye(128, dtype=np.float32)
    in_maps = []
    for c in range(NCORES):
        jsel = np.zeros((1, 8), np.int32)
        for d in range(8):
            jsel[0, d] = (c + d) % NCORES
        in_maps.append({
            "feats_all": feats,
            "feats_my": feats[c * RP : (c + 1) * RP],
            "ident_in": ident,
            "jsel_in": jsel,
        })
    res = run_bass_kernel_spmd(
        nc, in_maps, core_ids=list(range(NCORES)), trace=_trace
    )
    out = np.empty((N, N), dtype=np.float32)
    res_g = [res.results[c]["g_out"] for c in range(NCORES)]  # [RP, ND*RP]
    for c in range(NCORES):
        g = res_g[c]
        for d in range(ND):
            j = (c + d) % NCORES
            blk = g[:, d * RP : (d + 1) * RP]
            if d == 0:
                # kernel computed upper triangle only; mirror below diag
                full = np.triu(blk) + np.triu(blk, 1).T
                out[c * RP : (c + 1) * RP, c * RP : (c + 1) * RP] = full
            elif d == ND - 1:
                if c >= NCORES // 2:
                    continue
                # core c has triu of (c,j); core j has triu of (j,c) = tril^T
                blk2 = res_g[j][:, (ND - 1) * RP : ND * RP]
                full = np.triu(blk) + np.tril(blk2.T, -1)
                out[c * RP : (c + 1) * RP, j * RP : (j + 1) * RP] = full
                out[j * RP : (j + 1) * RP, c * RP : (c + 1) * RP] = full.T
            else:
                out[c * RP : (c + 1) * RP, j * RP : (j + 1) * RP] = blk
                out[j * RP : (j + 1) * RP, c * RP : (c + 1) * RP] = blk.T
    if _trace:
        return out, res
    return out


if __name__ == "__main__":
    inputs = {
        "feats": np.load("/tmp/feats.npy"),
        "kn": 10,
    }
    out = kernel(**inputs)
    print("out", out.shape, out.dtype, float(np.abs(out).max()))

